# revision 52
# baseline (speedup 1.0000x reference)
"""Multi-head attention block (qkv proj -> softmax attention -> out proj)
for B=2, N=2048, C=1024, H=16 heads of d=64, distributed over 8 NeuronCores.

Sharding: core c = (b, g) with b = c // 4 (batch), g = c % 4 (head group of
4 heads). Each core computes q/k/v for its 4 heads, full softmax attention,
and a partial output projection (its 256 input channels of w_proj). The
host sums the 4 per-batch partials and adds b_proj.

Pipeline design (per core): the ACT engine's exp stream is the roofline
(128 x [128,1024] activations ~ 1.15us each). Everything else (qkv
projection, v production, output projection, DMA) is emitted as
lower-priority PE/DVE filler that the scheduler slots under the exp
stream:
  - dedicated PSUM pools so score-tile rotation never serializes against
    qkv work: scores 2x[128,1024] (4 banks), PV accum 2x[128,512]
    (2 banks), misc matmul chains 2x[128,512] (2 banks).
  - input DMA is chunked so the first score matmuls (k/q of heads 0,1 for
    tokens 0:512) have their data within ~3us; the first exp fires ~10us.
  - ~40 dummy matmuls on scratch SBUF during the DMA window warm the PE
    HAM clock gate (1.2 -> 2.4 GHz) before real matmuls arrive.
  - per-step: one row-tiled score matmul pair (disjoint PE row groups,
    K=64 each), one exp, two PV matmuls; v_aug = [v | ones] makes the PV
    matmul emit the softmax denominator Z at psum partition 64.
  - softmax skips max-subtraction: scores*scale ~ N(0,1), safe in fp32.
  - output projection for i-chunk ic is emitted right after round (1, ic)
    so only the last chunk's projection remains in the tail.
"""

import sys
import types

import numpy as np
import ml_dtypes

B = 2
N = 2048
C = 1024
H = 16
D = 64
HL = H // 4          # heads per core = 4
SCALE = D ** -0.5
N_CORES = 8
KT = C // 128        # 8 contraction tiles
MT = N // 128        # 16 token tiles
BF = ml_dtypes.bfloat16

_cache = {}


def _install_ntff_hook():
    """Register the axon NTFF profiling hook that this image's antenv lacks
    (profiling degrades gracefully without it; needed for exec_time_ns)."""
    try:
        import antenv.axon_hooks  # noqa: F401
        return
    except ImportError:
        pass
    try:
        import antenv
        from trn_agent_boot.trn_boot import _ntff_profile_via_ctypes
    except ImportError:
        return
    mod = types.ModuleType("antenv.axon_hooks")
    _hook = [None]
    mod.set_axon_ntff_profile_hook = lambda h: _hook.__setitem__(0, h)
    mod.get_axon_ntff_profile_hook = lambda: _hook[0]
    sys.modules["antenv.axon_hooks"] = mod
    antenv.axon_hooks = mod
    try:
        mod.set_axon_ntff_profile_hook(
            _ntff_profile_via_ctypes("/opt/axon/libaxon_pjrt.so")
        )
    except Exception:
        pass


def _build_program(v_bias_nonzero: bool):
    from contextlib import ExitStack

    import concourse.bass as bass
    import concourse.tile as tile
    from concourse import bacc, mybir

    f32 = mybir.dt.float32
    bf16 = mybir.dt.bfloat16
    Exp = mybir.ActivationFunctionType.Exp
    add = mybir.AluOpType.add

    nc = bacc.Bacc("TRN2", target_bir_lowering=False, debug=False,
                   num_devices=N_CORES)

    # all inputs come pre-arranged by the host in SBUF layout [128, free]
    # so every DMA is a contiguous full-bandwidth transfer
    xTf_d = nc.dram_tensor("xTf", [128, KT * 512], bf16,
                           kind="ExternalInput").ap()
    xTr_d = nc.dram_tensor("xTr", [128, KT * 1536], bf16,
                           kind="ExternalInput").ap()
    wqkA_d = nc.dram_tensor("wqkA", [128, KT * 256], bf16,
                            kind="ExternalInput").ap()
    wqkB_d = nc.dram_tensor("wqkB", [128, KT * 256], bf16,
                            kind="ExternalInput").ap()
    wv_d = nc.dram_tensor("wv", [128, KT * 256], bf16,
                          kind="ExternalInput").ap()
    wp_d = nc.dram_tensor("wp", [128, 2 * C], bf16,
                          kind="ExternalInput").ap()
    bqk_d = nc.dram_tensor("bqk", [512, 1], f32, kind="ExternalInput").ap()
    bv_d = nc.dram_tensor("bv", [128, 4], f32, kind="ExternalInput").ap()
    y_d = nc.dram_tensor("y", [N, C], bf16, kind="ExternalOutput").ap()
    warm_d = nc.dram_tensor("warm", [1, 8], f32, kind="ExternalOutput").ap()

    with tile.TileContext(nc) as tc, ExitStack() as ctx:
        persist = ctx.enter_context(tc.tile_pool(name="persist", bufs=1))
        # PSUM budget (8 banks of 2KB): scores 2x[128,1024]f32 (4 banks) +
        # pv accumulators 2x[128,512] (2) + misc qkv/v/proj chains 2x[128,512]
        # (2). Dedicated pools keep the score rotation independent of the
        # qkv/proj chains so the exp stream starts as soon as k0/q0 land.
        s_pool = ctx.enter_context(
            tc.tile_pool(name="s", bufs=2, space="PSUM"))
        pv_pool = ctx.enter_context(
            tc.tile_pool(name="pv", bufs=2, space="PSUM"))
        mm_pool = ctx.enter_context(
            tc.tile_pool(name="mm", bufs=2, space="PSUM"))
        es_pool = ctx.enter_context(tc.tile_pool(name="es", bufs=4))
        z_pool = ctx.enter_context(tc.tile_pool(name="z", bufs=3))
        y_pool = ctx.enter_context(tc.tile_pool(name="ysb", bufs=4))
        zd_pool = ctx.enter_context(
            tc.tile_pool(name="zd", bufs=4, space="DRAM"))

        xT = persist.tile([128, KT, N], bf16)
        wqkA = persist.tile([128, KT, 256], bf16)
        wqkB = persist.tile([128, KT, 256], bf16)
        wv = persist.tile([128, KT, 256], bf16)
        wp = persist.tile([128, 2, C], bf16)
        bq = persist.tile([128, 4], f32)
        bv = persist.tile([128, 4], f32) if v_bias_nonzero else None
        # q/k activations split into per-(dim-tile, token-chunk) tiles so the
        # scheduler releases attention matmuls as soon as each chunk lands
        qkT = [[persist.tile([128, 512], bf16, name=f"qkT{nt}_{mc}")
                for mc in range(4)] for nt in range(4)]
        v_sb = persist.tile([128, MT, HL * 65], bf16)
        out_sb = persist.tile([128, 2, N], bf16)
        warm_sb = persist.tile([1, 8], f32)
        scratch = persist.tile([128, 128], bf16)
        ones64 = persist.tile([128, 64], f32)

        def dma_sb(q, dst_ap, src, src_w, kt0, nkt, c0=0, c1=None):
            """One DMA from an SBUF-layout dram tensor (per-kt row width
            src_w) covering kt tiles [kt0, kt0+nkt) x src cols [c0:c1)."""
            c1 = src_w if c1 is None else c1
            q.dma_start(
                dst_ap,
                src.rearrange("p (t c) -> p t c", c=src_w)
                [:, kt0:kt0 + nkt, c0:c1])

        # Input DMAs ordered by consumption deadline, spread over the three
        # DMA queues (sync/scalar/gpsimd -- the only ones with queues;
        # scalar's is free until the exp stream starts). Each dma_start
        # costs ~1us setup + transfer at ~100GB/s per queue, serial per
        # queue, so the critical set is few, large, contiguous transfers.
        # Phase A gates the first score matmuls: bq, wqkA (q01|k01), and
        # xT cols 0:512 (~1.5MB; kt0's chunks lead each queue).
        with tc.high_priority():
            # bqk[512,1] -> [128 partitions, 4 tiles]
            nc.sync.dma_start(bq[:],
                              bqk_d.rearrange("(t p) o -> p (t o)", p=128))
            if v_bias_nonzero:
                # bv[128, 4]: col h = bias of head h (d at p%64, doubled
                # across both partition halves)
                nc.scalar.dma_start(bv[:], bv_d[:])
            dma_sb(nc.sync, xT[:, 0:2, 0:512], xTf_d, 512, 0, 2)
            dma_sb(nc.scalar, xT[:, 2:4, 0:512], xTf_d, 512, 2, 2)
            dma_sb(nc.gpsimd, wqkA[:, 0:4, :], wqkA_d, 256, 0, 4)
            dma_sb(nc.sync, xT[:, 4:6, 0:512], xTf_d, 512, 4, 2)
            dma_sb(nc.scalar, xT[:, 6:8, 0:512], xTf_d, 512, 6, 2)
            dma_sb(nc.gpsimd, wqkA[:, 4:8, :], wqkA_d, 256, 4, 4)

            # warm-up exp: pulls the ACT table load off the critical path
            nc.vector.memset(warm_sb[:], 0.0)
            nc.scalar.activation(warm_sb[:], warm_sb[:], Exp)
            nc.sync.dma_start(warm_d[:], warm_sb[:])
            # ones columns of v_aug (per head, col 64 of each 65-col group)
            ones_ap = v_sb[:].rearrange("p mt (h c) -> p (mt h) c", c=65)
            nc.vector.memset(ones_ap[:, :, 64:65], 1.0)
            nc.vector.memset(ones64[:], 1.0)

            nc.vector.memset(scratch[:], 0.0)

        # Phase B: wv (gates the v chains from ~step 0 of round 0) and
        # xT cols 512:1024 (k chunk 1, scores from step 2; v tiles 4-7).
        dma_sb(nc.gpsimd, wv[:, 0:4, :], wv_d, 256, 0, 4)
        dma_sb(nc.sync, xT[:, 0:2, 512:1024], xTr_d, 1536, 0, 2, 0, 512)
        dma_sb(nc.scalar, xT[:, 4:6, 512:1024], xTr_d, 1536, 4, 2, 0, 512)
        dma_sb(nc.gpsimd, wv[:, 4:8, :], wv_d, 256, 4, 4)
        dma_sb(nc.sync, xT[:, 2:4, 512:1024], xTr_d, 1536, 2, 2, 0, 512)
        dma_sb(nc.scalar, xT[:, 6:8, 512:1024], xTr_d, 1536, 6, 2, 0, 512)
        # Phase C: xT cols 1024:1536 (k chunk 2 / v tiles 8-11)
        dma_sb(nc.gpsimd, xT[:, 0:2, 1024:1536], xTr_d, 1536, 0, 2, 512,
               1024)
        dma_sb(nc.sync, xT[:, 2:4, 1024:1536], xTr_d, 1536, 2, 2, 512, 1024)
        dma_sb(nc.scalar, xT[:, 4:6, 1024:1536], xTr_d, 1536, 4, 2, 512,
               1024)
        dma_sb(nc.gpsimd, xT[:, 6:8, 1024:1536], xTr_d, 1536, 6, 2, 512,
               1024)
        # Phase D: xT cols 1536:2048, wqkB (q23/k23, rounds 4-7), wp
        # (projection, from round 4 on)
        dma_sb(nc.sync, xT[:, 0:2, 1536:2048], xTr_d, 1536, 0, 2, 1024, 1536)
        dma_sb(nc.scalar, xT[:, 4:6, 1536:2048], xTr_d, 1536, 4, 2, 1024,
               1536)
        dma_sb(nc.gpsimd, wqkB[:, 0:4, :], wqkB_d, 256, 0, 4)
        dma_sb(nc.sync, xT[:, 2:4, 1536:2048], xTr_d, 1536, 2, 2, 1024, 1536)
        dma_sb(nc.scalar, xT[:, 6:8, 1536:2048], xTr_d, 1536, 6, 2, 1024,
               1536)
        dma_sb(nc.gpsimd, wqkB[:, 4:8, :], wqkB_d, 256, 4, 4)
        dma_sb(nc.gpsimd, wp[:], wp_d, C, 0, 2)

        # wqkA holds [q01|k01], wqkB holds [q23|k23]; nt 0..3 =
        # q01,q23,k01,k23 as before
        W_OFF = {0: (0, 0), 2: (0, 128), 1: (1, 256), 3: (1, 384)}

        def qk_block(nt, mcs=range(4)):
            half, off = W_OFF[nt]
            wt = wqkA if half == 0 else wqkB
            for mc in mcs:
                ps = mm_pool.tile([128, 512], f32, tag="mm",
                                  name=f"qk{nt}_{mc}")
                for kt in range(KT):
                    nc.tensor.matmul(
                        ps[:],
                        lhsT=wt[:, kt, off % 256:off % 256 + 128],
                        rhs=xT[:, kt, mc * 512:(mc + 1) * 512],
                        start=(kt == 0), stop=(kt == KT - 1))
                nc.vector.tensor_scalar(
                    out=qkT[nt][mc][:], in0=ps[:],
                    scalar1=bq[:, off // 128:off // 128 + 1],
                    scalar2=None, op0=add)

        def v_block(mts):
            for mt in mts:
                ps = mm_pool.tile([128, 256], f32, tag="mm", name=f"v{mt}")
                for kt in range(KT):
                    nc.tensor.matmul(
                        ps[:],
                        lhsT=xT[:, kt, mt * 128:(mt + 1) * 128],
                        rhs=wv[:, kt, :],
                        start=(kt == 0), stop=(kt == KT - 1))
                # v_aug per head = [v | ones]: the PV matmul then puts v at
                # psum partitions 0..63 and the denominator Z at partition 64
                dst = v_sb[:, mt, :].rearrange("p (h c) -> p h c", c=65)
                nc.vector.tensor_copy(
                    dst[:, :, 0:64], ps[:].rearrange("p (h c) -> p h c",
                                                     c=64))

        NG = MT

        def s_group(step):
            rnd, jt = step // NG, step % NG
            hp, ic = rnd // 4, rnd % 4
            ss = s_pool.tile([128, 1024], f32, tag="s",
                             name=f"s{hp}_{ic}_{jt}")
            for hh in range(2):
                po = hh * 64
                nc.tensor.matmul(
                    ss[:, hh * 512:(hh + 1) * 512],
                    lhsT=qkT[2 + hp][jt // 4][
                        po:po + 64, (jt % 4) * 128:(jt % 4 + 1) * 128],
                    rhs=qkT[hp][ic][po:po + 64, :],
                    start=True, stop=True)
            return ss

        def pv_normalize(hp, ic, pvs, fast=False):
            # in the tail, head 1's chain (which ends in a cross-partition
            # DMA) is the critical path -- start it first
            for hh in ((1, 0) if fast else (0, 1)):
                # release the pv psum slot quickly with a single copy, then
                # run the whole normalize chain from SBUF off-critical-path.
                # Even head: data at partitions 0:64, Z at 64. Odd head:
                # data at 64:128, Z at 63. The Z broadcast goes through a
                # DRAM round-trip normally; in the tail (fast=True) a K=1
                # fp32 matmul on the then-idle PE does it with ~5us less
                # latency.
                pv = pvs[hh]
                zb = z_pool.tile([64, 512], f32, tag="zb")
                if fast:
                    # tail path: ACT (idle after the last exp) lifts the Z
                    # row to SBUF, a K=1 fp32 matmul broadcasts it, and the
                    # normalize multiply reads the pv psum directly -- no
                    # oa round-trip, ~4us less latency.
                    oa = z_pool.tile([128, 512], f32, tag="oa")
                    nc.scalar.copy(oa[64:65, :], pv[64:65, :])
                    zp = s_pool.tile([128, 1024], f32, tag="s",
                                     name=f"zbc{hp}_{ic}_{hh}")
                    nc.tensor.matmul(zp[0:64, 0:512],
                                     lhsT=ones64[64:65, :],
                                     rhs=oa[64:65, :],
                                     start=True, stop=True)
                    nc.vector.reciprocal_approx_fast(zb[:], zp[0:64, 0:512])
                    src = pv
                else:
                    oa = z_pool.tile([128, 512], f32, tag="oa")
                    nc.vector.tensor_copy(oa[:], pv[:])
                    zd = zd_pool.tile([1, 512], f32, tag="zd")
                    nc.sync.dma_start(zd[:], oa[64:65, :])
                    zbz = z_pool.tile([64, 512], f32, tag="zbz")
                    nc.sync.dma_start(
                        zbz[:], zd[0:1, :].to_broadcast([64, 512]))
                    nc.vector.reciprocal_approx_fast(zb[:], zbz[:])
                    src = oa
                if hh == 0:
                    dst = out_sb[0:64, hp, ic * 512:(ic + 1) * 512]
                else:
                    dst = z_pool.tile([64, 512], bf16, tag="o1")
                nc.vector.tensor_mul(dst, src[0:64, :], zb[:])
                if v_bias_nonzero:
                    h = 2 * hp + hh
                    nc.vector.tensor_scalar(
                        out=dst, in0=dst, scalar1=bv[0:64, h:h + 1],
                        scalar2=None, op0=add)
                if hh == 1:
                    # cross-partition move to out_sb[64:128]; the scalar
                    # DMA queue is idle in the tail
                    q = nc.scalar if fast else nc.sync
                    q.dma_start(
                        out_sb[64:128, hp, ic * 512:(ic + 1) * 512],
                        dst[:])

        out_q = [nc.sync, nc.gpsimd]

        def proj_block(its):
            for it in its:
                for oc in range(2):
                    ps = mm_pool.tile([128, 512], f32, tag="mm",
                                      name=f"y{it}_{oc}")
                    for ct in range(2):
                        nc.tensor.matmul(
                            ps[:],
                            lhsT=out_sb[:, ct, it * 128:(it + 1) * 128],
                            rhs=wp[:, ct, oc * 512:(oc + 1) * 512],
                            start=(ct == 0), stop=(ct == 1))
                    ysb = y_pool.tile([128, 512], bf16, tag="y")
                    nc.vector.tensor_copy(ysb[:], ps[:])
                    out_q[(2 * it + oc) % 2].dma_start(
                        y_d[it * 128:(it + 1) * 128,
                            oc * 512:(oc + 1) * 512],
                        ysb[:])

        def proj_tail():
            # Last i-chunk: its ct=0 half (heads 0,1; ready since round
            # (0,3)) runs during the final normalize latency; ct=1 lands
            # as soon as out_sb ct1 is written. Two waves over 4 psum
            # slots (mm + the now-free pv pool); drains split DVE/ACT;
            # y DMAs on the idle scalar queue plus sync/gpsimd.
            tq = [nc.scalar, nc.sync, nc.gpsimd]
            for wave in ((12, 13), (14, 15)):
                units = [(it, oc) for it in wave for oc in range(2)]
                pss = []
                for k, (it, oc) in enumerate(units):
                    pool, tag = ((mm_pool, "mm") if k % 2 == 0
                                 else (pv_pool, "pv"))
                    ps = pool.tile([128, 512], f32, tag=tag,
                                   name=f"yt{it}_{oc}")
                    nc.tensor.matmul(
                        ps[:], lhsT=out_sb[:, 0, it * 128:(it + 1) * 128],
                        rhs=wp[:, 0, oc * 512:(oc + 1) * 512],
                        start=True, stop=False)
                    pss.append(ps)
                for k, (it, oc) in enumerate(units):
                    ps = pss[k]
                    nc.tensor.matmul(
                        ps[:], lhsT=out_sb[:, 1, it * 128:(it + 1) * 128],
                        rhs=wp[:, 1, oc * 512:(oc + 1) * 512],
                        start=False, stop=True)
                    ysb = y_pool.tile([128, 512], bf16, tag="y")
                    if k % 2 == 1:
                        nc.scalar.copy(ysb[:], ps[:])
                    else:
                        nc.vector.tensor_copy(ysb[:], ps[:])
                    tq[k % 3].dma_start(
                        y_d[it * 128:(it + 1) * 128,
                            oc * 512:(oc + 1) * 512],
                        ysb[:])

        # Critical path to the first exp: k and q of heads 0,1 for tokens
        # 0:512 (j-tiles 0-3, i-chunk 0). Dummy matmuls interleave with the
        # DMA-paced chain so the PE HAM activity window stays dense and the
        # clock gate releases (1.2 -> 2.4 GHz) before the main work: the
        # HAM only un-throttles after ~3.4us of gap-free PE activity.
        warm_ps = pv_pool.tile([128, 512], f32, tag="pv", name="hamwarm")

        def dummies(n):
            for i in range(n):
                lw = scratch[:, 0:64] if i % 2 == 0 else scratch[:, 64:128]
                nc.tensor.matmul(warm_ps[0:64, 0:128], lhsT=lw,
                                 rhs=scratch[:], start=True, stop=True)

        def qk_block_warm(nt, mc, nd):
            half, off = W_OFF[nt]
            wt = wqkA if half == 0 else wqkB
            ps = mm_pool.tile([128, 512], f32, tag="mm", name=f"qkw{nt}")
            for kt in range(KT):
                nc.tensor.matmul(
                    ps[:],
                    lhsT=wt[:, kt, off % 256:off % 256 + 128],
                    rhs=xT[:, kt, mc * 512:(mc + 1) * 512],
                    start=(kt == 0), stop=(kt == KT - 1))
                dummies(nd)
            nc.vector.tensor_scalar(
                out=qkT[nt][mc][:], in0=ps[:],
                scalar1=bq[:, off // 128:off // 128 + 1],
                scalar2=None, op0=add)

        dummies(40)
        qk_block_warm(2, 0, 5)     # k chunk 0 for heads 0,1
        qk_block_warm(0, 0, 3)     # q chunk 0 for heads 0,1
        # Everything else is PE filler under the exp stream, ordered by
        # when round 0 needs it: v tiles jt feed PV step jt, k chunk c
        # feeds score steps 4c.., q chunks feed later rounds.
        # deadline order (in exp steps): v[jt] -> step jt, k chunk c ->
        # step 4c-2 (scores run LOOK ahead), q0[ic] -> step 16ic-2,
        # k3/q1 -> rounds 4-7
        with tc.high_priority(offset=-20000):
            v_block(range(0, 2))
            qk_block(2, [1])
            v_block(range(2, 6))
            qk_block(2, [2])
            v_block(range(6, 10))
            qk_block(2, [3])
            v_block(range(10, 13))
            qk_block(0, [1])
            v_block(range(13, 16))
            qk_block(0, [2])
            qk_block(0, [3])
            qk_block(3, [0])
            qk_block(1, [0])
            qk_block(3, [1])
            qk_block(3, [2])
            qk_block(3, [3])
            qk_block(1, [1])
            qk_block(1, [2, 3])

        # One flat software pipeline across all 8 (hp, ic) rounds: scores
        # stay LOOK groups ahead of the exp stream so the in-order PE queue
        # never head-of-line-blocks it.
        NSTEP = 8 * NG
        LOOK = 2
        def pv_step(pvs, hp, jt, es):
            for hh in range(2):
                h = 2 * hp + hh
                nc.tensor.matmul(
                    pvs[hh][0:65, :],
                    lhsT=v_sb[:, jt, h * 65:(h + 1) * 65],
                    rhs=es[:, hh * 512:(hh + 1) * 512],
                    start=(jt == 0), stop=(jt == MT - 1))

        with tc.high_priority():
            ss_q = {i: s_group(i) for i in range(LOOK)}
            pvs = None
            es0 = None
            for st in range(NSTEP):
                rnd, jt = st // NG, st % NG
                hp, ic = rnd // 4, rnd % 4
                if jt == 0:
                    pvs = [pv_pool.tile([128, 512], f32, tag="pv",
                                        name=f"pv{hp}_{ic}_{i}")
                           for i in range(2)]
                es = es_pool.tile([128, 1024], bf16, tag="es")
                nc.scalar.activation(es[:], ss_q[st % LOOK][:], Exp,
                                     scale=SCALE)
                if st + LOOK < NSTEP:
                    ss_q[st % LOOK] = s_group(st + LOOK)
                # jt==0's PV matmuls wait on the pv slot being drained
                # (previous round's oa copy); defer their PE-queue slot by
                # one step so they don't head-of-line-block the next exp's
                # scores at the round boundary
                if jt == 0:
                    es0 = es
                else:
                    if jt == 1:
                        pv_step(pvs, hp, 0, es0)
                    pv_step(pvs, hp, jt, es)
                if jt == NG - 1:
                    pv_normalize(hp, ic, pvs, fast=(st == NSTEP - 1))
                    if hp == 1:
                        # both head-pairs of i-chunk ic done: its output
                        # projection becomes pure filler -- except the last
                        # chunk, which IS the tail critical path
                        if ic == 3:
                            proj_tail()
                        else:
                            with tc.high_priority(offset=-15000):
                                proj_block(range(4 * ic, 4 * ic + 4))

    nc.compile()
    return nc


def _prep_inputs(x, w_qkv, b_qkv, w_proj):
    """Build the 8 per-core input maps (host-side shard + transpose + cast)."""
    w3 = w_qkv.reshape(C, 3, H, D)
    b3 = b_qkv.reshape(3, H, D)
    in_maps = []
    for c in range(N_CORES):
        b, g = divmod(c, 4)
        hs = slice(g * HL, (g + 1) * HL)
        wq = w3[:, 0, hs, :].reshape(C, 256)
        wk = w3[:, 1, hs, :].reshape(C, 256)
        wvl = w3[:, 2, hs, :].reshape(C, 256)
        bqh = b3[0, hs, :].reshape(256)
        bkh = b3[1, hs, :].reshape(256)
        bvh = b3[2, hs, :].reshape(256)
        # q/k transposed layout: head pair (2j, 2j+1) shares an SBUF tile
        # with partition offsets 0/64. All matrices are pre-arranged in
        # SBUF layout [128, kt*cols] so device DMAs are contiguous.
        def sb_layout(m, cols):
            return np.ascontiguousarray(
                m.reshape(KT, 128, cols).transpose(1, 0, 2)
                .reshape(128, KT * cols)).astype(BF)

        xt = x[b].T                       # [C, N]
        in_maps.append({
            "xTf": sb_layout(xt[:, 0:512], 512),
            "xTr": sb_layout(xt[:, 512:], N - 512),
            "wqkA": sb_layout(
                np.concatenate([wq[:, :128], wk[:, :128]], axis=1), 256),
            "wqkB": sb_layout(
                np.concatenate([wq[:, 128:], wk[:, 128:]], axis=1), 256),
            "wv": sb_layout(wvl, 256),
            "wp": np.ascontiguousarray(
                w_proj[g * 256:(g + 1) * 256, :].reshape(2, 128, C)
                .transpose(1, 0, 2).reshape(128, 2 * C)).astype(BF),
            "bqk": np.concatenate(
                [bqh[:128], bkh[:128], bqh[128:], bkh[128:]])
                .reshape(512, 1).astype(np.float32),
            "bv": np.ascontiguousarray(
                      np.tile(bvh.reshape(4, 64).T, (2, 1)))
                    .astype(np.float32),
        })
    return in_maps


def _get_program(v_bias_nonzero: bool):
    key = ("prog", v_bias_nonzero)
    if key not in _cache:
        _install_ntff_hook()
        _cache[key] = _build_program(v_bias_nonzero)
    return _cache[key]


def run(x, w_qkv, b_qkv, w_proj, b_proj, trace=False, trace_kwargs=None):
    from concourse import bass_utils
    bass_utils.upload_artifacts = lambda tmpdir: tmpdir  # no cloud upload

    x = np.asarray(x, dtype=np.float32)
    w_qkv = np.asarray(w_qkv, dtype=np.float32)
    b_qkv = np.asarray(b_qkv, dtype=np.float32)
    w_proj = np.asarray(w_proj, dtype=np.float32)
    b_proj = np.asarray(b_proj, dtype=np.float32)

    v_bias_nonzero = bool(np.any(b_qkv.reshape(3, H, D)[2] != 0.0))
    nc = _get_program(v_bias_nonzero)
    in_maps = _prep_inputs(x, w_qkv, b_qkv, w_proj)
    res = bass_utils.run_bass_kernel_spmd(
        nc, in_maps, list(range(N_CORES)), trace=trace,
        **(trace_kwargs or {}))

    out = np.zeros((B, N, C), dtype=np.float32)
    for b in range(B):
        acc = np.zeros((N, C), dtype=np.float32)
        for g in range(4):
            acc += np.asarray(res.results[b * 4 + g]["y"],
                              dtype=np.float32)
        out[b] = acc + b_proj
    return out, res


def kernel(x, w_qkv, b_qkv, w_proj, b_proj):
    out, _ = run(x, w_qkv, b_qkv, w_proj, b_proj, trace=False)
    return out


# revision 53
# speedup vs baseline: 1.0120x; 1.0120x over previous
"""Multi-head attention block (qkv proj -> softmax attention -> out proj)
for B=2, N=2048, C=1024, H=16 heads of d=64, distributed over 8 NeuronCores.

Sharding: core c = (b, g) with b = c // 4 (batch), g = c % 4 (head group of
4 heads). Each core computes q/k/v for its 4 heads, full softmax attention,
and a partial output projection (its 256 input channels of w_proj). The
host sums the 4 per-batch partials and adds b_proj.

Pipeline design (per core): the ACT engine's exp stream is the roofline
(128 x [128,1024] activations ~ 1.15us each). Everything else (qkv
projection, v production, output projection, DMA) is emitted as
lower-priority PE/DVE filler that the scheduler slots under the exp
stream:
  - dedicated PSUM pools so score-tile rotation never serializes against
    qkv work: scores 2x[128,1024] (4 banks), PV accum 2x[128,512]
    (2 banks), misc matmul chains 2x[128,512] (2 banks).
  - input DMA is chunked so the first score matmuls (k/q of heads 0,1 for
    tokens 0:512) have their data within ~3us; the first exp fires ~10us.
  - ~40 dummy matmuls on scratch SBUF during the DMA window warm the PE
    HAM clock gate (1.2 -> 2.4 GHz) before real matmuls arrive.
  - per-step: one row-tiled score matmul pair (disjoint PE row groups,
    K=64 each), one exp, two PV matmuls; v_aug = [v | ones] makes the PV
    matmul emit the softmax denominator Z at psum partition 64.
  - softmax skips max-subtraction: scores*scale ~ N(0,1), safe in fp32.
  - output projection for i-chunk ic is emitted right after round (1, ic)
    so only the last chunk's projection remains in the tail.
"""

import sys
import types

import numpy as np
import ml_dtypes

B = 2
N = 2048
C = 1024
H = 16
D = 64
HL = H // 4          # heads per core = 4
SCALE = D ** -0.5
N_CORES = 8
KT = C // 128        # 8 contraction tiles
MT = N // 128        # 16 token tiles
BF = ml_dtypes.bfloat16

_cache = {}


def _install_ntff_hook():
    """Register the axon NTFF profiling hook that this image's antenv lacks
    (profiling degrades gracefully without it; needed for exec_time_ns)."""
    try:
        import antenv.axon_hooks  # noqa: F401
        return
    except ImportError:
        pass
    try:
        import antenv
        from trn_agent_boot.trn_boot import _ntff_profile_via_ctypes
    except ImportError:
        return
    mod = types.ModuleType("antenv.axon_hooks")
    _hook = [None]
    mod.set_axon_ntff_profile_hook = lambda h: _hook.__setitem__(0, h)
    mod.get_axon_ntff_profile_hook = lambda: _hook[0]
    sys.modules["antenv.axon_hooks"] = mod
    antenv.axon_hooks = mod
    try:
        mod.set_axon_ntff_profile_hook(
            _ntff_profile_via_ctypes("/opt/axon/libaxon_pjrt.so")
        )
    except Exception:
        pass


def _build_program(v_bias_nonzero: bool):
    from contextlib import ExitStack

    import concourse.bass as bass
    import concourse.tile as tile
    from concourse import bacc, mybir

    f32 = mybir.dt.float32
    bf16 = mybir.dt.bfloat16
    Exp = mybir.ActivationFunctionType.Exp
    add = mybir.AluOpType.add

    nc = bacc.Bacc("TRN2", target_bir_lowering=False, debug=False,
                   num_devices=N_CORES)

    # all inputs come pre-arranged by the host in SBUF layout [128, free]
    # so every DMA is a contiguous full-bandwidth transfer
    xTf_d = nc.dram_tensor("xTf", [128, KT * 512], bf16,
                           kind="ExternalInput").ap()
    xTr_d = nc.dram_tensor("xTr", [128, KT * 1536], bf16,
                           kind="ExternalInput").ap()
    wqkA_d = nc.dram_tensor("wqkA", [128, KT * 256], bf16,
                            kind="ExternalInput").ap()
    wqkB_d = nc.dram_tensor("wqkB", [128, KT * 256], bf16,
                            kind="ExternalInput").ap()
    wv_d = nc.dram_tensor("wv", [128, KT * 256], bf16,
                          kind="ExternalInput").ap()
    wp_d = nc.dram_tensor("wp", [128, 2 * C], bf16,
                          kind="ExternalInput").ap()
    bqk_d = nc.dram_tensor("bqk", [512, 1], f32, kind="ExternalInput").ap()
    bv_d = nc.dram_tensor("bv", [128, 4], f32, kind="ExternalInput").ap()
    y_d = nc.dram_tensor("y", [N, C], bf16, kind="ExternalOutput").ap()
    warm_d = nc.dram_tensor("warm", [1, 8], f32, kind="ExternalOutput").ap()

    with tile.TileContext(nc) as tc, ExitStack() as ctx:
        persist = ctx.enter_context(tc.tile_pool(name="persist", bufs=1))
        # PSUM budget (8 banks of 2KB): scores 2x[128,1024]f32 (4 banks) +
        # pv accumulators 2x[128,512] (2) + misc qkv/v/proj chains 2x[128,512]
        # (2). Dedicated pools keep the score rotation independent of the
        # qkv/proj chains so the exp stream starts as soon as k0/q0 land.
        s_pool = ctx.enter_context(
            tc.tile_pool(name="s", bufs=2, space="PSUM"))
        pv_pool = ctx.enter_context(
            tc.tile_pool(name="pv", bufs=2, space="PSUM"))
        mm_pool = ctx.enter_context(
            tc.tile_pool(name="mm", bufs=2, space="PSUM"))
        es_pool = ctx.enter_context(tc.tile_pool(name="es", bufs=4))
        z_pool = ctx.enter_context(tc.tile_pool(name="z", bufs=3))
        y_pool = ctx.enter_context(tc.tile_pool(name="ysb", bufs=4))
        zd_pool = ctx.enter_context(
            tc.tile_pool(name="zd", bufs=4, space="DRAM"))

        xT = persist.tile([128, KT, N], bf16)
        wqkA = persist.tile([128, KT, 256], bf16)
        wqkB = persist.tile([128, KT, 256], bf16)
        wv = persist.tile([128, KT, 256], bf16)
        wp = persist.tile([128, 2, C], bf16)
        bq = persist.tile([128, 4], f32)
        bv = persist.tile([128, 4], f32) if v_bias_nonzero else None
        # q/k activations split into per-(dim-tile, token-chunk) tiles so the
        # scheduler releases attention matmuls as soon as each chunk lands
        qkT = [[persist.tile([128, 512], bf16, name=f"qkT{nt}_{mc}")
                for mc in range(4)] for nt in range(4)]
        v_sb = persist.tile([128, MT, HL * 65], bf16)
        out_sb = persist.tile([128, 2, N], bf16)
        warm_sb = persist.tile([1, 8], f32)
        scratch = persist.tile([128, 128], bf16)
        ones64 = persist.tile([128, 64], f32)

        def dma_sb(q, dst_ap, src, src_w, kt0, nkt, c0=0, c1=None):
            """One DMA from an SBUF-layout dram tensor (per-kt row width
            src_w) covering kt tiles [kt0, kt0+nkt) x src cols [c0:c1)."""
            c1 = src_w if c1 is None else c1
            q.dma_start(
                dst_ap,
                src.rearrange("p (t c) -> p t c", c=src_w)
                [:, kt0:kt0 + nkt, c0:c1])

        # Input DMAs ordered by consumption deadline, spread over the three
        # DMA queues (sync/scalar/gpsimd -- the only ones with queues;
        # scalar's is free until the exp stream starts). Each dma_start
        # costs ~1us setup + transfer at ~100GB/s per queue, serial per
        # queue, so the critical set is few, large, contiguous transfers.
        # Phase A gates the first score matmuls: bq, wqkA (q01|k01), and
        # xT cols 0:512 (~1.5MB; kt0's chunks lead each queue).
        with tc.high_priority():
            # bqk[512,1] -> [128 partitions, 4 tiles]
            nc.sync.dma_start(bq[:],
                              bqk_d.rearrange("(t p) o -> p (t o)", p=128))
            if v_bias_nonzero:
                # bv[128, 4]: col h = bias of head h (d at p%64, doubled
                # across both partition halves)
                nc.scalar.dma_start(bv[:], bv_d[:])
            dma_sb(nc.sync, xT[:, 0:2, 0:512], xTf_d, 512, 0, 2)
            dma_sb(nc.scalar, xT[:, 2:4, 0:512], xTf_d, 512, 2, 2)
            dma_sb(nc.gpsimd, wqkA[:, 0:4, :], wqkA_d, 256, 0, 4)
            dma_sb(nc.sync, xT[:, 4:6, 0:512], xTf_d, 512, 4, 2)
            dma_sb(nc.scalar, xT[:, 6:8, 0:512], xTf_d, 512, 6, 2)
            dma_sb(nc.gpsimd, wqkA[:, 4:8, :], wqkA_d, 256, 4, 4)

            # warm-up exp: pulls the ACT table load off the critical path
            nc.vector.memset(warm_sb[:], 0.0)
            nc.scalar.activation(warm_sb[:], warm_sb[:], Exp)
            nc.sync.dma_start(warm_d[:], warm_sb[:])
            # ones columns of v_aug (per head, col 64 of each 65-col group)
            ones_ap = v_sb[:].rearrange("p mt (h c) -> p (mt h) c", c=65)
            nc.vector.memset(ones_ap[:, :, 64:65], 1.0)
            nc.vector.memset(ones64[:], 1.0)

            nc.vector.memset(scratch[:], 0.0)

        # Phase B: wv (gates the v chains from ~step 0 of round 0) and
        # xT cols 512:1024 (k chunk 1, scores from step 2; v tiles 4-7).
        dma_sb(nc.gpsimd, wv[:, 0:4, :], wv_d, 256, 0, 4)
        dma_sb(nc.sync, xT[:, 0:2, 512:1024], xTr_d, 1536, 0, 2, 0, 512)
        dma_sb(nc.scalar, xT[:, 4:6, 512:1024], xTr_d, 1536, 4, 2, 0, 512)
        dma_sb(nc.gpsimd, wv[:, 4:8, :], wv_d, 256, 4, 4)
        dma_sb(nc.sync, xT[:, 2:4, 512:1024], xTr_d, 1536, 2, 2, 0, 512)
        dma_sb(nc.scalar, xT[:, 6:8, 512:1024], xTr_d, 1536, 6, 2, 0, 512)
        # Phase C: xT cols 1024:1536 (k chunk 2 / v tiles 8-11)
        dma_sb(nc.gpsimd, xT[:, 0:2, 1024:1536], xTr_d, 1536, 0, 2, 512,
               1024)
        dma_sb(nc.sync, xT[:, 2:4, 1024:1536], xTr_d, 1536, 2, 2, 512, 1024)
        dma_sb(nc.scalar, xT[:, 4:6, 1024:1536], xTr_d, 1536, 4, 2, 512,
               1024)
        dma_sb(nc.gpsimd, xT[:, 6:8, 1024:1536], xTr_d, 1536, 6, 2, 512,
               1024)
        # Phase D: xT cols 1536:2048, wqkB (q23/k23, rounds 4-7), wp
        # (projection, from round 4 on)
        dma_sb(nc.sync, xT[:, 0:2, 1536:2048], xTr_d, 1536, 0, 2, 1024, 1536)
        dma_sb(nc.scalar, xT[:, 4:6, 1536:2048], xTr_d, 1536, 4, 2, 1024,
               1536)
        dma_sb(nc.gpsimd, wqkB[:, 0:4, :], wqkB_d, 256, 0, 4)
        dma_sb(nc.sync, xT[:, 2:4, 1536:2048], xTr_d, 1536, 2, 2, 1024, 1536)
        dma_sb(nc.scalar, xT[:, 6:8, 1536:2048], xTr_d, 1536, 6, 2, 1024,
               1536)
        dma_sb(nc.gpsimd, wqkB[:, 4:8, :], wqkB_d, 256, 4, 4)
        dma_sb(nc.gpsimd, wp[:], wp_d, C, 0, 2)

        # wqkA holds [q01|k01], wqkB holds [q23|k23]; nt 0..3 =
        # q01,q23,k01,k23 as before
        W_OFF = {0: (0, 0), 2: (0, 128), 1: (1, 256), 3: (1, 384)}

        def qk_block(nt, mcs=range(4)):
            half, off = W_OFF[nt]
            wt = wqkA if half == 0 else wqkB
            for mc in mcs:
                ps = mm_pool.tile([128, 512], f32, tag="mm",
                                  name=f"qk{nt}_{mc}")
                for kt in range(KT):
                    nc.tensor.matmul(
                        ps[:],
                        lhsT=wt[:, kt, off % 256:off % 256 + 128],
                        rhs=xT[:, kt, mc * 512:(mc + 1) * 512],
                        start=(kt == 0), stop=(kt == KT - 1))
                nc.vector.tensor_scalar(
                    out=qkT[nt][mc][:], in0=ps[:],
                    scalar1=bq[:, off // 128:off // 128 + 1],
                    scalar2=None, op0=add)

        def v_block(mts):
            for mt in mts:
                ps = mm_pool.tile([128, 256], f32, tag="mm", name=f"v{mt}")
                for kt in range(KT):
                    nc.tensor.matmul(
                        ps[:],
                        lhsT=xT[:, kt, mt * 128:(mt + 1) * 128],
                        rhs=wv[:, kt, :],
                        start=(kt == 0), stop=(kt == KT - 1))
                # v_aug per head = [v | ones]: the PV matmul then puts v at
                # psum partitions 0..63 and the denominator Z at partition 64
                dst = v_sb[:, mt, :].rearrange("p (h c) -> p h c", c=65)
                nc.vector.tensor_copy(
                    dst[:, :, 0:64], ps[:].rearrange("p (h c) -> p h c",
                                                     c=64))

        NG = MT

        def s_group(step):
            rnd, jt = step // NG, step % NG
            hp, ic = rnd // 4, rnd % 4
            ss = s_pool.tile([128, 1024], f32, tag="s",
                             name=f"s{hp}_{ic}_{jt}")
            for hh in range(2):
                po = hh * 64
                nc.tensor.matmul(
                    ss[:, hh * 512:(hh + 1) * 512],
                    lhsT=qkT[2 + hp][jt // 4][
                        po:po + 64, (jt % 4) * 128:(jt % 4 + 1) * 128],
                    rhs=qkT[hp][ic][po:po + 64, :],
                    start=True, stop=True)
            return ss

        def pv_normalize(hp, ic, pvs, fast=False):
            # in the tail, head 1's chain (which ends in a cross-partition
            # DMA) is the critical path -- start it first
            for hh in ((1, 0) if fast else (0, 1)):
                # release the pv psum slot quickly with a single copy, then
                # run the whole normalize chain from SBUF off-critical-path.
                # Even head: data at partitions 0:64, Z at 64. Odd head:
                # data at 64:128, Z at 63. The Z broadcast goes through a
                # DRAM round-trip normally; in the tail (fast=True) a K=1
                # fp32 matmul on the then-idle PE does it with ~5us less
                # latency.
                pv = pvs[hh]
                zb = z_pool.tile([64, 512], f32, tag="zb")
                if fast:
                    # tail path: ACT (idle after the last exp) lifts the Z
                    # row to SBUF, a K=1 fp32 matmul broadcasts it, and the
                    # normalize multiply reads the pv psum directly -- no
                    # oa round-trip, ~4us less latency.
                    oa = z_pool.tile([128, 512], f32, tag="oa")
                    nc.scalar.copy(oa[64:65, :], pv[64:65, :])
                    zp = s_pool.tile([128, 1024], f32, tag="s",
                                     name=f"zbc{hp}_{ic}_{hh}")
                    nc.tensor.matmul(zp[0:64, 0:512],
                                     lhsT=ones64[64:65, :],
                                     rhs=oa[64:65, :],
                                     start=True, stop=True)
                    nc.vector.reciprocal_approx_fast(zb[:], zp[0:64, 0:512])
                    src = pv
                else:
                    oa = z_pool.tile([128, 512], f32, tag="oa")
                    nc.vector.tensor_copy(oa[:], pv[:])
                    zd = zd_pool.tile([1, 512], f32, tag="zd")
                    nc.sync.dma_start(zd[:], oa[64:65, :])
                    zbz = z_pool.tile([64, 512], f32, tag="zbz")
                    nc.sync.dma_start(
                        zbz[:], zd[0:1, :].to_broadcast([64, 512]))
                    nc.vector.reciprocal_approx_fast(zb[:], zbz[:])
                    src = oa
                if hh == 0:
                    dst = out_sb[0:64, hp, ic * 512:(ic + 1) * 512]
                else:
                    dst = z_pool.tile([64, 512], bf16, tag="o1")
                nc.vector.tensor_mul(dst, src[0:64, :], zb[:])
                if v_bias_nonzero:
                    h = 2 * hp + hh
                    nc.vector.tensor_scalar(
                        out=dst, in0=dst, scalar1=bv[0:64, h:h + 1],
                        scalar2=None, op0=add)
                if hh == 1:
                    # cross-partition move to out_sb[64:128]; the scalar
                    # DMA queue is idle in the tail
                    q = nc.scalar if fast else nc.sync
                    q.dma_start(
                        out_sb[64:128, hp, ic * 512:(ic + 1) * 512],
                        dst[:])

        out_q = [nc.sync, nc.gpsimd]

        def proj_block(its):
            for it in its:
                for oc in range(2):
                    ps = mm_pool.tile([128, 512], f32, tag="mm",
                                      name=f"y{it}_{oc}")
                    for ct in range(2):
                        nc.tensor.matmul(
                            ps[:],
                            lhsT=out_sb[:, ct, it * 128:(it + 1) * 128],
                            rhs=wp[:, ct, oc * 512:(oc + 1) * 512],
                            start=(ct == 0), stop=(ct == 1))
                    ysb = y_pool.tile([128, 512], bf16, tag="y")
                    nc.vector.tensor_copy(ysb[:], ps[:])
                    out_q[(2 * it + oc) % 2].dma_start(
                        y_d[it * 128:(it + 1) * 128,
                            oc * 512:(oc + 1) * 512],
                        ysb[:])

        def proj_tail():
            # Last i-chunk: its ct=0 half (heads 0,1; ready since round
            # (0,3)) runs during the final normalize latency; ct=1 lands
            # as soon as out_sb ct1 is written. Two waves over 4 psum
            # slots (mm + the now-free pv pool); drains split DVE/ACT;
            # y DMAs on the idle scalar queue plus sync/gpsimd.
            tq = [nc.scalar, nc.sync, nc.gpsimd]
            for wave in ((12, 13), (14, 15)):
                units = [(it, oc) for it in wave for oc in range(2)]
                pss = []
                for k, (it, oc) in enumerate(units):
                    pool, tag = ((mm_pool, "mm") if k % 2 == 0
                                 else (pv_pool, "pv"))
                    ps = pool.tile([128, 512], f32, tag=tag,
                                   name=f"yt{it}_{oc}")
                    nc.tensor.matmul(
                        ps[:], lhsT=out_sb[:, 0, it * 128:(it + 1) * 128],
                        rhs=wp[:, 0, oc * 512:(oc + 1) * 512],
                        start=True, stop=False)
                    pss.append(ps)
                for k, (it, oc) in enumerate(units):
                    ps = pss[k]
                    nc.tensor.matmul(
                        ps[:], lhsT=out_sb[:, 1, it * 128:(it + 1) * 128],
                        rhs=wp[:, 1, oc * 512:(oc + 1) * 512],
                        start=False, stop=True)
                    ysb = y_pool.tile([128, 512], bf16, tag="y")
                    if k % 2 == 1:
                        nc.scalar.copy(ysb[:], ps[:])
                    else:
                        nc.vector.tensor_copy(ysb[:], ps[:])
                    tq[k % 3].dma_start(
                        y_d[it * 128:(it + 1) * 128,
                            oc * 512:(oc + 1) * 512],
                        ysb[:])

        # Critical path to the first exp: k and q of heads 0,1 for tokens
        # 0:512 (j-tiles 0-3, i-chunk 0). Dummy matmuls interleave with the
        # DMA-paced chain so the PE HAM activity window stays dense and the
        # clock gate releases (1.2 -> 2.4 GHz) before the main work: the
        # HAM only un-throttles after ~3.4us of gap-free PE activity.
        warm_ps = pv_pool.tile([128, 512], f32, tag="pv", name="hamwarm")

        def dummies(n):
            for i in range(n):
                lw = scratch[:, 0:64] if i % 2 == 0 else scratch[:, 64:128]
                nc.tensor.matmul(warm_ps[0:64, 0:128], lhsT=lw,
                                 rhs=scratch[:], start=True, stop=True)

        def qk_block_warm(nt, mc):
            half, off = W_OFF[nt]
            wt = wqkA if half == 0 else wqkB
            ps = mm_pool.tile([128, 512], f32, tag="mm", name=f"qkw{nt}")
            for kt in range(KT):
                nc.tensor.matmul(
                    ps[:],
                    lhsT=wt[:, kt, off % 256:off % 256 + 128],
                    rhs=xT[:, kt, mc * 512:(mc + 1) * 512],
                    start=(kt == 0), stop=(kt == KT - 1))
                if nt == 2 and 3 <= kt <= 6:
                    dummies(4)   # bridge the phase-A supply stall
            nc.vector.tensor_scalar(
                out=qkT[nt][mc][:], in0=ps[:],
                scalar1=bq[:, off // 128:off // 128 + 1],
                scalar2=None, op0=add)

        dummies(58)
        qk_block_warm(2, 0)        # k chunk 0 for heads 0,1
        qk_block_warm(0, 0)        # q chunk 0 for heads 0,1
        # Everything else is PE filler under the exp stream, ordered by
        # when round 0 needs it: v tiles jt feed PV step jt, k chunk c
        # feeds score steps 4c.., q chunks feed later rounds.
        # deadline order (in exp steps): v[jt] -> step jt, k chunk c ->
        # step 4c-2 (scores run LOOK ahead), q0[ic] -> step 16ic-2,
        # k3/q1 -> rounds 4-7
        with tc.high_priority(offset=-20000):
            v_block(range(0, 2))
            qk_block(2, [1])
            v_block(range(2, 6))
            qk_block(2, [2])
            v_block(range(6, 10))
            qk_block(2, [3])
            v_block(range(10, 13))
            qk_block(0, [1])
            v_block(range(13, 16))
            qk_block(0, [2])
            qk_block(0, [3])
            qk_block(3, [0])
            qk_block(1, [0])
            qk_block(3, [1])
            qk_block(3, [2])
            qk_block(3, [3])
            qk_block(1, [1])
            qk_block(1, [2, 3])

        # One flat software pipeline across all 8 (hp, ic) rounds: scores
        # stay LOOK groups ahead of the exp stream so the in-order PE queue
        # never head-of-line-blocks it.
        NSTEP = 8 * NG
        LOOK = 2
        def pv_step(pvs, hp, jt, es):
            for hh in range(2):
                h = 2 * hp + hh
                nc.tensor.matmul(
                    pvs[hh][0:65, :],
                    lhsT=v_sb[:, jt, h * 65:(h + 1) * 65],
                    rhs=es[:, hh * 512:(hh + 1) * 512],
                    start=(jt == 0), stop=(jt == MT - 1))

        with tc.high_priority():
            ss_q = {i: s_group(i) for i in range(LOOK)}
            pvs = None
            es0 = None
            for st in range(NSTEP):
                rnd, jt = st // NG, st % NG
                hp, ic = rnd // 4, rnd % 4
                if jt == 0:
                    pvs = [pv_pool.tile([128, 512], f32, tag="pv",
                                        name=f"pv{hp}_{ic}_{i}")
                           for i in range(2)]
                es = es_pool.tile([128, 1024], bf16, tag="es")
                nc.scalar.activation(es[:], ss_q[st % LOOK][:], Exp,
                                     scale=SCALE)
                if st + LOOK < NSTEP:
                    ss_q[st % LOOK] = s_group(st + LOOK)
                # jt==0's PV matmuls wait on the pv slot being drained
                # (previous round's oa copy); defer their PE-queue slot by
                # one step so they don't head-of-line-block the next exp's
                # scores at the round boundary
                if jt == 0:
                    es0 = es
                else:
                    if jt == 1:
                        pv_step(pvs, hp, 0, es0)
                    pv_step(pvs, hp, jt, es)
                if jt == NG - 1:
                    pv_normalize(hp, ic, pvs, fast=(st == NSTEP - 1))
                    if hp == 1:
                        # both head-pairs of i-chunk ic done: its output
                        # projection becomes pure filler -- except the last
                        # chunk, which IS the tail critical path
                        if ic == 3:
                            proj_tail()
                        else:
                            with tc.high_priority(offset=-15000):
                                proj_block(range(4 * ic, 4 * ic + 4))

    nc.compile()
    return nc


def _prep_inputs(x, w_qkv, b_qkv, w_proj):
    """Build the 8 per-core input maps (host-side shard + transpose + cast)."""
    w3 = w_qkv.reshape(C, 3, H, D)
    b3 = b_qkv.reshape(3, H, D)
    in_maps = []
    for c in range(N_CORES):
        b, g = divmod(c, 4)
        hs = slice(g * HL, (g + 1) * HL)
        wq = w3[:, 0, hs, :].reshape(C, 256)
        wk = w3[:, 1, hs, :].reshape(C, 256)
        wvl = w3[:, 2, hs, :].reshape(C, 256)
        bqh = b3[0, hs, :].reshape(256)
        bkh = b3[1, hs, :].reshape(256)
        bvh = b3[2, hs, :].reshape(256)
        # q/k transposed layout: head pair (2j, 2j+1) shares an SBUF tile
        # with partition offsets 0/64. All matrices are pre-arranged in
        # SBUF layout [128, kt*cols] so device DMAs are contiguous.
        def sb_layout(m, cols):
            return np.ascontiguousarray(
                m.reshape(KT, 128, cols).transpose(1, 0, 2)
                .reshape(128, KT * cols)).astype(BF)

        xt = x[b].T                       # [C, N]
        in_maps.append({
            "xTf": sb_layout(xt[:, 0:512], 512),
            "xTr": sb_layout(xt[:, 512:], N - 512),
            "wqkA": sb_layout(
                np.concatenate([wq[:, :128], wk[:, :128]], axis=1), 256),
            "wqkB": sb_layout(
                np.concatenate([wq[:, 128:], wk[:, 128:]], axis=1), 256),
            "wv": sb_layout(wvl, 256),
            "wp": np.ascontiguousarray(
                w_proj[g * 256:(g + 1) * 256, :].reshape(2, 128, C)
                .transpose(1, 0, 2).reshape(128, 2 * C)).astype(BF),
            "bqk": np.concatenate(
                [bqh[:128], bkh[:128], bqh[128:], bkh[128:]])
                .reshape(512, 1).astype(np.float32),
            "bv": np.ascontiguousarray(
                      np.tile(bvh.reshape(4, 64).T, (2, 1)))
                    .astype(np.float32),
        })
    return in_maps


def _get_program(v_bias_nonzero: bool):
    key = ("prog", v_bias_nonzero)
    if key not in _cache:
        _install_ntff_hook()
        _cache[key] = _build_program(v_bias_nonzero)
    return _cache[key]


def run(x, w_qkv, b_qkv, w_proj, b_proj, trace=False, trace_kwargs=None):
    from concourse import bass_utils
    bass_utils.upload_artifacts = lambda tmpdir: tmpdir  # no cloud upload

    x = np.asarray(x, dtype=np.float32)
    w_qkv = np.asarray(w_qkv, dtype=np.float32)
    b_qkv = np.asarray(b_qkv, dtype=np.float32)
    w_proj = np.asarray(w_proj, dtype=np.float32)
    b_proj = np.asarray(b_proj, dtype=np.float32)

    v_bias_nonzero = bool(np.any(b_qkv.reshape(3, H, D)[2] != 0.0))
    nc = _get_program(v_bias_nonzero)
    in_maps = _prep_inputs(x, w_qkv, b_qkv, w_proj)
    res = bass_utils.run_bass_kernel_spmd(
        nc, in_maps, list(range(N_CORES)), trace=trace,
        **(trace_kwargs or {}))

    out = np.zeros((B, N, C), dtype=np.float32)
    for b in range(B):
        acc = np.zeros((N, C), dtype=np.float32)
        for g in range(4):
            acc += np.asarray(res.results[b * 4 + g]["y"],
                              dtype=np.float32)
        out[b] = acc + b_proj
    return out, res


def kernel(x, w_qkv, b_qkv, w_proj, b_proj):
    out, _ = run(x, w_qkv, b_qkv, w_proj, b_proj, trace=False)
    return out


# revision 54
# speedup vs baseline: 1.0131x; 1.0010x over previous
"""Multi-head attention block (qkv proj -> softmax attention -> out proj)
for B=2, N=2048, C=1024, H=16 heads of d=64, distributed over 8 NeuronCores.

Sharding: core c = (b, g) with b = c // 4 (batch), g = c % 4 (head group of
4 heads). Each core computes q/k/v for its 4 heads, full softmax attention,
and a partial output projection (its 256 input channels of w_proj). The
host sums the 4 per-batch partials and adds b_proj.

Pipeline design (per core): the ACT engine's exp stream is the roofline
(128 x [128,1024] activations ~ 1.15us each). Everything else (qkv
projection, v production, output projection, DMA) is emitted as
lower-priority PE/DVE filler that the scheduler slots under the exp
stream:
  - dedicated PSUM pools so score-tile rotation never serializes against
    qkv work: scores 2x[128,1024] (4 banks), PV accum 2x[128,512]
    (2 banks), misc matmul chains 2x[128,512] (2 banks).
  - input DMA is chunked so the first score matmuls (k/q of heads 0,1 for
    tokens 0:512) have their data within ~3us; the first exp fires ~10us.
  - ~40 dummy matmuls on scratch SBUF during the DMA window warm the PE
    HAM clock gate (1.2 -> 2.4 GHz) before real matmuls arrive.
  - per-step: one row-tiled score matmul pair (disjoint PE row groups,
    K=64 each), one exp, two PV matmuls; v_aug = [v | ones] makes the PV
    matmul emit the softmax denominator Z at psum partition 64.
  - softmax skips max-subtraction: scores*scale ~ N(0,1), safe in fp32.
  - output projection for i-chunk ic is emitted right after round (1, ic)
    so only the last chunk's projection remains in the tail.
"""

import sys
import types

import numpy as np
import ml_dtypes

B = 2
N = 2048
C = 1024
H = 16
D = 64
HL = H // 4          # heads per core = 4
SCALE = D ** -0.5
N_CORES = 8
KT = C // 128        # 8 contraction tiles
MT = N // 128        # 16 token tiles
BF = ml_dtypes.bfloat16

_cache = {}


def _install_ntff_hook():
    """Register the axon NTFF profiling hook that this image's antenv lacks
    (profiling degrades gracefully without it; needed for exec_time_ns)."""
    try:
        import antenv.axon_hooks  # noqa: F401
        return
    except ImportError:
        pass
    try:
        import antenv
        from trn_agent_boot.trn_boot import _ntff_profile_via_ctypes
    except ImportError:
        return
    mod = types.ModuleType("antenv.axon_hooks")
    _hook = [None]
    mod.set_axon_ntff_profile_hook = lambda h: _hook.__setitem__(0, h)
    mod.get_axon_ntff_profile_hook = lambda: _hook[0]
    sys.modules["antenv.axon_hooks"] = mod
    antenv.axon_hooks = mod
    try:
        mod.set_axon_ntff_profile_hook(
            _ntff_profile_via_ctypes("/opt/axon/libaxon_pjrt.so")
        )
    except Exception:
        pass


def _build_program(v_bias_nonzero: bool):
    from contextlib import ExitStack

    import concourse.bass as bass
    import concourse.tile as tile
    from concourse import bacc, mybir

    f32 = mybir.dt.float32
    bf16 = mybir.dt.bfloat16
    Exp = mybir.ActivationFunctionType.Exp
    add = mybir.AluOpType.add

    nc = bacc.Bacc("TRN2", target_bir_lowering=False, debug=False,
                   num_devices=N_CORES)

    # all inputs come pre-arranged by the host in SBUF layout [128, free]
    # so every DMA is a contiguous full-bandwidth transfer
    xTf_d = nc.dram_tensor("xTf", [128, KT * 512], bf16,
                           kind="ExternalInput").ap()
    xTr_d = nc.dram_tensor("xTr", [128, KT * 1536], bf16,
                           kind="ExternalInput").ap()
    wqkA_d = nc.dram_tensor("wqkA", [128, KT * 256], bf16,
                            kind="ExternalInput").ap()
    wqkB_d = nc.dram_tensor("wqkB", [128, KT * 256], bf16,
                            kind="ExternalInput").ap()
    wv_d = nc.dram_tensor("wv", [128, KT * 256], bf16,
                          kind="ExternalInput").ap()
    wp_d = nc.dram_tensor("wp", [128, 2 * C], bf16,
                          kind="ExternalInput").ap()
    bqk_d = nc.dram_tensor("bqk", [512, 1], f32, kind="ExternalInput").ap()
    bv_d = nc.dram_tensor("bv", [128, 4], f32, kind="ExternalInput").ap()
    y_d = nc.dram_tensor("y", [N, C], bf16, kind="ExternalOutput").ap()
    warm_d = nc.dram_tensor("warm", [1, 8], f32, kind="ExternalOutput").ap()

    with tile.TileContext(nc) as tc, ExitStack() as ctx:
        persist = ctx.enter_context(tc.tile_pool(name="persist", bufs=1))
        # PSUM budget (8 banks of 2KB): scores 2x[128,1024]f32 (4 banks) +
        # pv accumulators 2x[128,512] (2) + misc qkv/v/proj chains 2x[128,512]
        # (2). Dedicated pools keep the score rotation independent of the
        # qkv/proj chains so the exp stream starts as soon as k0/q0 land.
        s_pool = ctx.enter_context(
            tc.tile_pool(name="s", bufs=2, space="PSUM"))
        pv_pool = ctx.enter_context(
            tc.tile_pool(name="pv", bufs=2, space="PSUM"))
        mm_pool = ctx.enter_context(
            tc.tile_pool(name="mm", bufs=2, space="PSUM"))
        es_pool = ctx.enter_context(tc.tile_pool(name="es", bufs=4))
        z_pool = ctx.enter_context(tc.tile_pool(name="z", bufs=3))
        y_pool = ctx.enter_context(tc.tile_pool(name="ysb", bufs=4))
        zd_pool = ctx.enter_context(
            tc.tile_pool(name="zd", bufs=4, space="DRAM"))

        xT = persist.tile([128, KT, N], bf16)
        wqkA = persist.tile([128, KT, 256], bf16)
        wqkB = persist.tile([128, KT, 256], bf16)
        wv = persist.tile([128, KT, 256], bf16)
        wp = persist.tile([128, 2, C], bf16)
        bq = persist.tile([128, 4], f32)
        bv = persist.tile([128, 4], f32) if v_bias_nonzero else None
        # q/k activations split into per-(dim-tile, token-chunk) tiles so the
        # scheduler releases attention matmuls as soon as each chunk lands
        qkT = [[persist.tile([128, 512], bf16, name=f"qkT{nt}_{mc}")
                for mc in range(4)] for nt in range(4)]
        v_sb = persist.tile([128, MT, HL * 65], bf16)
        out_sb = persist.tile([128, 2, N], bf16)
        warm_sb = persist.tile([1, 8], f32)
        scratch = persist.tile([128, 128], bf16)
        ones64 = persist.tile([128, 64], f32)

        def dma_sb(q, dst_ap, src, src_w, kt0, nkt, c0=0, c1=None):
            """One DMA from an SBUF-layout dram tensor (per-kt row width
            src_w) covering kt tiles [kt0, kt0+nkt) x src cols [c0:c1)."""
            c1 = src_w if c1 is None else c1
            q.dma_start(
                dst_ap,
                src.rearrange("p (t c) -> p t c", c=src_w)
                [:, kt0:kt0 + nkt, c0:c1])

        # Input DMAs ordered by consumption deadline, spread over the three
        # DMA queues (sync/scalar/gpsimd -- the only ones with queues;
        # scalar's is free until the exp stream starts). Each dma_start
        # costs ~1us setup + transfer at ~100GB/s per queue, serial per
        # queue, so the critical set is few, large, contiguous transfers.
        # Phase A gates the first score matmuls: bq, wqkA (q01|k01), and
        # xT cols 0:512 (~1.5MB; kt0's chunks lead each queue).
        with tc.high_priority():
            # bqk[512,1] -> [128 partitions, 4 tiles]
            nc.sync.dma_start(bq[:],
                              bqk_d.rearrange("(t p) o -> p (t o)", p=128))
            if v_bias_nonzero:
                # bv[128, 4]: col h = bias of head h (d at p%64, doubled
                # across both partition halves)
                nc.scalar.dma_start(bv[:], bv_d[:])
            dma_sb(nc.sync, xT[:, 0:2, 0:512], xTf_d, 512, 0, 2)
            dma_sb(nc.scalar, xT[:, 2:4, 0:512], xTf_d, 512, 2, 2)
            dma_sb(nc.gpsimd, wqkA[:, 0:4, :], wqkA_d, 256, 0, 4)
            dma_sb(nc.sync, xT[:, 4:6, 0:512], xTf_d, 512, 4, 2)
            dma_sb(nc.scalar, xT[:, 6:8, 0:512], xTf_d, 512, 6, 2)
            dma_sb(nc.gpsimd, wqkA[:, 4:8, :], wqkA_d, 256, 4, 4)

            # warm-up exp: pulls the ACT table load off the critical path
            nc.vector.memset(warm_sb[:], 0.0)
            nc.scalar.activation(warm_sb[:], warm_sb[:], Exp)
            nc.sync.dma_start(warm_d[:], warm_sb[:])
            # ones columns of v_aug (per head, col 64 of each 65-col group)
            ones_ap = v_sb[:].rearrange("p mt (h c) -> p (mt h) c", c=65)
            nc.vector.memset(ones_ap[:, :, 64:65], 1.0)
            nc.vector.memset(ones64[:], 1.0)

            nc.vector.memset(scratch[:], 0.0)

        # Phase B: wv (gates the v chains from ~step 0 of round 0) and
        # xT cols 512:1024 (k chunk 1, scores from step 2; v tiles 4-7).
        dma_sb(nc.gpsimd, wv[:, 0:4, :], wv_d, 256, 0, 4)
        dma_sb(nc.sync, xT[:, 0:2, 512:1024], xTr_d, 1536, 0, 2, 0, 512)
        dma_sb(nc.scalar, xT[:, 4:6, 512:1024], xTr_d, 1536, 4, 2, 0, 512)
        dma_sb(nc.gpsimd, wv[:, 4:8, :], wv_d, 256, 4, 4)
        dma_sb(nc.sync, xT[:, 2:4, 512:1024], xTr_d, 1536, 2, 2, 0, 512)
        dma_sb(nc.scalar, xT[:, 6:8, 512:1024], xTr_d, 1536, 6, 2, 0, 512)
        # Phase C: xT cols 1024:1536 (k chunk 2 / v tiles 8-11)
        dma_sb(nc.gpsimd, xT[:, 0:2, 1024:1536], xTr_d, 1536, 0, 2, 512,
               1024)
        dma_sb(nc.sync, xT[:, 2:4, 1024:1536], xTr_d, 1536, 2, 2, 512, 1024)
        dma_sb(nc.scalar, xT[:, 4:6, 1024:1536], xTr_d, 1536, 4, 2, 512,
               1024)
        dma_sb(nc.gpsimd, xT[:, 6:8, 1024:1536], xTr_d, 1536, 6, 2, 512,
               1024)
        # Phase D: xT cols 1536:2048, wqkB (q23/k23, rounds 4-7), wp
        # (projection, from round 4 on)
        dma_sb(nc.sync, xT[:, 0:2, 1536:2048], xTr_d, 1536, 0, 2, 1024, 1536)
        dma_sb(nc.scalar, xT[:, 4:6, 1536:2048], xTr_d, 1536, 4, 2, 1024,
               1536)
        dma_sb(nc.gpsimd, wqkB[:, 0:4, :], wqkB_d, 256, 0, 4)
        dma_sb(nc.sync, xT[:, 2:4, 1536:2048], xTr_d, 1536, 2, 2, 1024, 1536)
        dma_sb(nc.scalar, xT[:, 6:8, 1536:2048], xTr_d, 1536, 6, 2, 1024,
               1536)
        dma_sb(nc.gpsimd, wqkB[:, 4:8, :], wqkB_d, 256, 4, 4)
        dma_sb(nc.gpsimd, wp[:], wp_d, C, 0, 2)

        # wqkA holds [q01|k01], wqkB holds [q23|k23]; nt 0..3 =
        # q01,q23,k01,k23 as before
        W_OFF = {0: (0, 0), 2: (0, 128), 1: (1, 256), 3: (1, 384)}

        def qk_block(nt, mcs=range(4)):
            half, off = W_OFF[nt]
            wt = wqkA if half == 0 else wqkB
            for mc in mcs:
                ps = mm_pool.tile([128, 512], f32, tag="mm",
                                  name=f"qk{nt}_{mc}")
                for kt in range(KT):
                    nc.tensor.matmul(
                        ps[:],
                        lhsT=wt[:, kt, off % 256:off % 256 + 128],
                        rhs=xT[:, kt, mc * 512:(mc + 1) * 512],
                        start=(kt == 0), stop=(kt == KT - 1))
                nc.vector.tensor_scalar(
                    out=qkT[nt][mc][:], in0=ps[:],
                    scalar1=bq[:, off // 128:off // 128 + 1],
                    scalar2=None, op0=add)

        def v_block(mts):
            for mt in mts:
                ps = mm_pool.tile([128, 256], f32, tag="mm", name=f"v{mt}")
                for kt in range(KT):
                    nc.tensor.matmul(
                        ps[:],
                        lhsT=xT[:, kt, mt * 128:(mt + 1) * 128],
                        rhs=wv[:, kt, :],
                        start=(kt == 0), stop=(kt == KT - 1))
                # v_aug per head = [v | ones]: the PV matmul then puts v at
                # psum partitions 0..63 and the denominator Z at partition 64
                dst = v_sb[:, mt, :].rearrange("p (h c) -> p h c", c=65)
                nc.vector.tensor_copy(
                    dst[:, :, 0:64], ps[:].rearrange("p (h c) -> p h c",
                                                     c=64))

        NG = MT

        def s_group(step):
            rnd, jt = step // NG, step % NG
            hp, ic = rnd // 4, rnd % 4
            ss = s_pool.tile([128, 1024], f32, tag="s",
                             name=f"s{hp}_{ic}_{jt}")
            for hh in range(2):
                po = hh * 64
                nc.tensor.matmul(
                    ss[:, hh * 512:(hh + 1) * 512],
                    lhsT=qkT[2 + hp][jt // 4][
                        po:po + 64, (jt % 4) * 128:(jt % 4 + 1) * 128],
                    rhs=qkT[hp][ic][po:po + 64, :],
                    start=True, stop=True)
            return ss

        def pv_normalize(hp, ic, pvs, fast=False):
            # in the tail, head 1's chain (which ends in a cross-partition
            # DMA) is the critical path -- start it first
            for hh in ((1, 0) if fast else (0, 1)):
                # release the pv psum slot quickly with a single copy, then
                # run the whole normalize chain from SBUF off-critical-path.
                # Even head: data at partitions 0:64, Z at 64. Odd head:
                # data at 64:128, Z at 63. The Z broadcast goes through a
                # DRAM round-trip normally; in the tail (fast=True) a K=1
                # fp32 matmul on the then-idle PE does it with ~5us less
                # latency.
                pv = pvs[hh]
                zb = z_pool.tile([64, 512], f32, tag="zb")
                if fast:
                    # tail path: ACT (idle after the last exp) lifts the Z
                    # row to SBUF, a K=1 fp32 matmul broadcasts it, and the
                    # normalize multiply reads the pv psum directly -- no
                    # oa round-trip, ~4us less latency.
                    oa = z_pool.tile([128, 512], f32, tag="oa")
                    nc.scalar.copy(oa[64:65, :], pv[64:65, :])
                    zp = s_pool.tile([128, 1024], f32, tag="s",
                                     name=f"zbc{hp}_{ic}_{hh}")
                    nc.tensor.matmul(zp[0:64, 0:512],
                                     lhsT=ones64[64:65, :],
                                     rhs=oa[64:65, :],
                                     start=True, stop=True)
                    nc.vector.reciprocal_approx_fast(zb[:], zp[0:64, 0:512])
                    src = pv
                else:
                    oa = z_pool.tile([128, 512], f32, tag="oa")
                    nc.vector.tensor_copy(oa[:], pv[:])
                    zd = zd_pool.tile([1, 512], f32, tag="zd")
                    nc.sync.dma_start(zd[:], oa[64:65, :])
                    zbz = z_pool.tile([64, 512], f32, tag="zbz")
                    nc.sync.dma_start(
                        zbz[:], zd[0:1, :].to_broadcast([64, 512]))
                    nc.vector.reciprocal_approx_fast(zb[:], zbz[:])
                    src = oa
                if hh == 0:
                    dst = out_sb[0:64, hp, ic * 512:(ic + 1) * 512]
                else:
                    dst = z_pool.tile([64, 512], bf16, tag="o1")
                nc.vector.tensor_mul(dst, src[0:64, :], zb[:])
                if v_bias_nonzero:
                    h = 2 * hp + hh
                    nc.vector.tensor_scalar(
                        out=dst, in0=dst, scalar1=bv[0:64, h:h + 1],
                        scalar2=None, op0=add)
                if hh == 1:
                    # cross-partition move to out_sb[64:128]; the scalar
                    # DMA queue is idle in the tail
                    q = nc.scalar if fast else nc.sync
                    q.dma_start(
                        out_sb[64:128, hp, ic * 512:(ic + 1) * 512],
                        dst[:])

        out_q = [nc.sync, nc.gpsimd]

        def proj_block(its):
            for it in its:
                for oc in range(2):
                    ps = mm_pool.tile([128, 512], f32, tag="mm",
                                      name=f"y{it}_{oc}")
                    for ct in range(2):
                        nc.tensor.matmul(
                            ps[:],
                            lhsT=out_sb[:, ct, it * 128:(it + 1) * 128],
                            rhs=wp[:, ct, oc * 512:(oc + 1) * 512],
                            start=(ct == 0), stop=(ct == 1))
                    ysb = y_pool.tile([128, 512], bf16, tag="y")
                    nc.vector.tensor_copy(ysb[:], ps[:])
                    out_q[(2 * it + oc) % 2].dma_start(
                        y_d[it * 128:(it + 1) * 128,
                            oc * 512:(oc + 1) * 512],
                        ysb[:])

        def proj_tail():
            # Last i-chunk: its ct=0 half (heads 0,1; ready since round
            # (0,3)) runs during the final normalize latency; ct=1 lands
            # as soon as out_sb ct1 is written. Two waves over 4 psum
            # slots (mm + the now-free pv pool); drains split DVE/ACT;
            # y DMAs on the idle scalar queue plus sync/gpsimd.
            tq = [nc.scalar, nc.sync, nc.gpsimd]
            for wave in ((12, 13), (14, 15)):
                units = [(it, oc) for it in wave for oc in range(2)]
                pss = []
                for k, (it, oc) in enumerate(units):
                    pool, tag = ((mm_pool, "mm") if k % 2 == 0
                                 else (pv_pool, "pv"))
                    ps = pool.tile([128, 512], f32, tag=tag,
                                   name=f"yt{it}_{oc}")
                    nc.tensor.matmul(
                        ps[:], lhsT=out_sb[:, 0, it * 128:(it + 1) * 128],
                        rhs=wp[:, 0, oc * 512:(oc + 1) * 512],
                        start=True, stop=False)
                    pss.append(ps)
                for k, (it, oc) in enumerate(units):
                    ps = pss[k]
                    nc.tensor.matmul(
                        ps[:], lhsT=out_sb[:, 1, it * 128:(it + 1) * 128],
                        rhs=wp[:, 1, oc * 512:(oc + 1) * 512],
                        start=False, stop=True)
                    ysb = y_pool.tile([128, 512], bf16, tag="y")
                    if k % 2 == 1:
                        nc.scalar.copy(ysb[:], ps[:])
                    else:
                        nc.vector.tensor_copy(ysb[:], ps[:])
                    tq[k % 3].dma_start(
                        y_d[it * 128:(it + 1) * 128,
                            oc * 512:(oc + 1) * 512],
                        ysb[:])

        # Critical path to the first exp: k and q of heads 0,1 for tokens
        # 0:512 (j-tiles 0-3, i-chunk 0).
        qk_block(2, [0])           # k chunk 0 for heads 0,1
        qk_block(0, [0])           # q chunk 0 for heads 0,1
        # Everything else is PE filler under the exp stream, ordered by
        # when round 0 needs it: v tiles jt feed PV step jt, k chunk c
        # feeds score steps 4c.., q chunks feed later rounds.
        # deadline order (in exp steps): v[jt] -> step jt, k chunk c ->
        # step 4c-2 (scores run LOOK ahead), q0[ic] -> step 16ic-2,
        # k3/q1 -> rounds 4-7
        with tc.high_priority(offset=-20000):
            v_block(range(0, 2))
            qk_block(2, [1])
            v_block(range(2, 6))
            qk_block(2, [2])
            v_block(range(6, 10))
            qk_block(2, [3])
            v_block(range(10, 13))
            qk_block(0, [1])
            v_block(range(13, 16))
            qk_block(0, [2])
            qk_block(0, [3])
            qk_block(3, [0])
            qk_block(1, [0])
            qk_block(3, [1])
            qk_block(3, [2])
            qk_block(3, [3])
            qk_block(1, [1])
            qk_block(1, [2, 3])

        # One flat software pipeline across all 8 (hp, ic) rounds: scores
        # stay LOOK groups ahead of the exp stream so the in-order PE queue
        # never head-of-line-blocks it.
        NSTEP = 8 * NG
        LOOK = 2
        def pv_step(pvs, hp, jt, es):
            for hh in range(2):
                h = 2 * hp + hh
                nc.tensor.matmul(
                    pvs[hh][0:65, :],
                    lhsT=v_sb[:, jt, h * 65:(h + 1) * 65],
                    rhs=es[:, hh * 512:(hh + 1) * 512],
                    start=(jt == 0), stop=(jt == MT - 1))

        with tc.high_priority():
            ss_q = {i: s_group(i) for i in range(LOOK)}
            pvs = None
            es0 = None
            for st in range(NSTEP):
                rnd, jt = st // NG, st % NG
                hp, ic = rnd // 4, rnd % 4
                if jt == 0:
                    pvs = [pv_pool.tile([128, 512], f32, tag="pv",
                                        name=f"pv{hp}_{ic}_{i}")
                           for i in range(2)]
                es = es_pool.tile([128, 1024], bf16, tag="es")
                nc.scalar.activation(es[:], ss_q[st % LOOK][:], Exp,
                                     scale=SCALE)
                if st + LOOK < NSTEP:
                    ss_q[st % LOOK] = s_group(st + LOOK)
                # jt==0's PV matmuls wait on the pv slot being drained
                # (previous round's oa copy); defer their PE-queue slot by
                # one step so they don't head-of-line-block the next exp's
                # scores at the round boundary
                if jt == 0:
                    es0 = es
                else:
                    if jt == 1:
                        pv_step(pvs, hp, 0, es0)
                    pv_step(pvs, hp, jt, es)
                if jt == NG - 1:
                    pv_normalize(hp, ic, pvs, fast=(st == NSTEP - 1))
                    if hp == 1:
                        # both head-pairs of i-chunk ic done: its output
                        # projection becomes pure filler -- except the last
                        # chunk, which IS the tail critical path
                        if ic == 3:
                            proj_tail()
                        else:
                            with tc.high_priority(offset=-15000):
                                proj_block(range(4 * ic, 4 * ic + 4))

    nc.compile()
    return nc


def _prep_inputs(x, w_qkv, b_qkv, w_proj):
    """Build the 8 per-core input maps (host-side shard + transpose + cast)."""
    w3 = w_qkv.reshape(C, 3, H, D)
    b3 = b_qkv.reshape(3, H, D)
    in_maps = []
    for c in range(N_CORES):
        b, g = divmod(c, 4)
        hs = slice(g * HL, (g + 1) * HL)
        wq = w3[:, 0, hs, :].reshape(C, 256)
        wk = w3[:, 1, hs, :].reshape(C, 256)
        wvl = w3[:, 2, hs, :].reshape(C, 256)
        bqh = b3[0, hs, :].reshape(256)
        bkh = b3[1, hs, :].reshape(256)
        bvh = b3[2, hs, :].reshape(256)
        # q/k transposed layout: head pair (2j, 2j+1) shares an SBUF tile
        # with partition offsets 0/64. All matrices are pre-arranged in
        # SBUF layout [128, kt*cols] so device DMAs are contiguous.
        def sb_layout(m, cols):
            return np.ascontiguousarray(
                m.reshape(KT, 128, cols).transpose(1, 0, 2)
                .reshape(128, KT * cols)).astype(BF)

        xt = x[b].T                       # [C, N]
        in_maps.append({
            "xTf": sb_layout(xt[:, 0:512], 512),
            "xTr": sb_layout(xt[:, 512:], N - 512),
            "wqkA": sb_layout(
                np.concatenate([wq[:, :128], wk[:, :128]], axis=1), 256),
            "wqkB": sb_layout(
                np.concatenate([wq[:, 128:], wk[:, 128:]], axis=1), 256),
            "wv": sb_layout(wvl, 256),
            "wp": np.ascontiguousarray(
                w_proj[g * 256:(g + 1) * 256, :].reshape(2, 128, C)
                .transpose(1, 0, 2).reshape(128, 2 * C)).astype(BF),
            "bqk": np.concatenate(
                [bqh[:128], bkh[:128], bqh[128:], bkh[128:]])
                .reshape(512, 1).astype(np.float32),
            "bv": np.ascontiguousarray(
                      np.tile(bvh.reshape(4, 64).T, (2, 1)))
                    .astype(np.float32),
        })
    return in_maps


def _get_program(v_bias_nonzero: bool):
    key = ("prog", v_bias_nonzero)
    if key not in _cache:
        _install_ntff_hook()
        _cache[key] = _build_program(v_bias_nonzero)
    return _cache[key]


def run(x, w_qkv, b_qkv, w_proj, b_proj, trace=False, trace_kwargs=None):
    from concourse import bass_utils
    bass_utils.upload_artifacts = lambda tmpdir: tmpdir  # no cloud upload

    x = np.asarray(x, dtype=np.float32)
    w_qkv = np.asarray(w_qkv, dtype=np.float32)
    b_qkv = np.asarray(b_qkv, dtype=np.float32)
    w_proj = np.asarray(w_proj, dtype=np.float32)
    b_proj = np.asarray(b_proj, dtype=np.float32)

    v_bias_nonzero = bool(np.any(b_qkv.reshape(3, H, D)[2] != 0.0))
    nc = _get_program(v_bias_nonzero)
    in_maps = _prep_inputs(x, w_qkv, b_qkv, w_proj)
    res = bass_utils.run_bass_kernel_spmd(
        nc, in_maps, list(range(N_CORES)), trace=trace,
        **(trace_kwargs or {}))

    out = np.zeros((B, N, C), dtype=np.float32)
    for b in range(B):
        acc = np.zeros((N, C), dtype=np.float32)
        for g in range(4):
            acc += np.asarray(res.results[b * 4 + g]["y"],
                              dtype=np.float32)
        out[b] = acc + b_proj
    return out, res


def kernel(x, w_qkv, b_qkv, w_proj, b_proj):
    out, _ = run(x, w_qkv, b_qkv, w_proj, b_proj, trace=False)
    return out


# revision 55
# speedup vs baseline: 1.0480x; 1.0345x over previous
"""Multi-head attention block (qkv proj -> softmax attention -> out proj)
for B=2, N=2048, C=1024, H=16 heads of d=64, distributed over 8 NeuronCores.

Sharding: core c = (b, g) with b = c // 4 (batch), g = c % 4 (head group of
4 heads). Each core computes q/k/v for its 4 heads, full softmax attention,
and a partial output projection (its 256 input channels of w_proj). The
host sums the 4 per-batch partials and adds b_proj.

Pipeline design (per core): the ACT engine's exp stream is the roofline
(128 x [128,1024] activations ~ 1.15us each). Everything else (qkv
projection, v production, output projection, DMA) is emitted as
lower-priority PE/DVE filler that the scheduler slots under the exp
stream:
  - dedicated PSUM pools so score-tile rotation never serializes against
    qkv work: scores 2x[128,1024] (4 banks), PV accum 2x[128,512]
    (2 banks), misc matmul chains 2x[128,512] (2 banks).
  - input DMA is chunked so the first score matmuls (k/q of heads 0,1 for
    tokens 0:512) have their data within ~3us; the first exp fires ~10us.
  - ~40 dummy matmuls on scratch SBUF during the DMA window warm the PE
    HAM clock gate (1.2 -> 2.4 GHz) before real matmuls arrive.
  - per-step: one row-tiled score matmul pair (disjoint PE row groups,
    K=64 each), one exp, two PV matmuls; v_aug = [v | ones] makes the PV
    matmul emit the softmax denominator Z at psum partition 64.
  - softmax skips max-subtraction: scores*scale ~ N(0,1), safe in fp32.
  - output projection for i-chunk ic is emitted right after round (1, ic)
    so only the last chunk's projection remains in the tail.
"""

import sys
import types

import numpy as np
import ml_dtypes

B = 2
N = 2048
C = 1024
H = 16
D = 64
HL = H // 4          # heads per core = 4
SCALE = D ** -0.5
N_CORES = 8
KT = C // 128        # 8 contraction tiles
MT = N // 128        # 16 token tiles
BF = ml_dtypes.bfloat16

_cache = {}


def _install_ntff_hook():
    """Register the axon NTFF profiling hook that this image's antenv lacks
    (profiling degrades gracefully without it; needed for exec_time_ns)."""
    try:
        import antenv.axon_hooks  # noqa: F401
        return
    except ImportError:
        pass
    try:
        import antenv
        from trn_agent_boot.trn_boot import _ntff_profile_via_ctypes
    except ImportError:
        return
    mod = types.ModuleType("antenv.axon_hooks")
    _hook = [None]
    mod.set_axon_ntff_profile_hook = lambda h: _hook.__setitem__(0, h)
    mod.get_axon_ntff_profile_hook = lambda: _hook[0]
    sys.modules["antenv.axon_hooks"] = mod
    antenv.axon_hooks = mod
    try:
        mod.set_axon_ntff_profile_hook(
            _ntff_profile_via_ctypes("/opt/axon/libaxon_pjrt.so")
        )
    except Exception:
        pass


def _build_program(v_bias_nonzero: bool):
    from contextlib import ExitStack

    import concourse.bass as bass
    import concourse.tile as tile
    from concourse import bacc, mybir

    f32 = mybir.dt.float32
    bf16 = mybir.dt.bfloat16
    Exp = mybir.ActivationFunctionType.Exp
    add = mybir.AluOpType.add

    nc = bacc.Bacc("TRN2", target_bir_lowering=False, debug=False,
                   num_devices=N_CORES)

    # all inputs come pre-arranged by the host in SBUF layout [128, free]
    # so every DMA is a contiguous full-bandwidth transfer
    xTf_d = nc.dram_tensor("xTf", [128, KT * 512], bf16,
                           kind="ExternalInput").ap()
    xTr_d = nc.dram_tensor("xTr", [128, KT * 1536], bf16,
                           kind="ExternalInput").ap()
    wqkA_d = nc.dram_tensor("wqkA", [128, KT * 256], bf16,
                            kind="ExternalInput").ap()
    wqkB_d = nc.dram_tensor("wqkB", [128, KT * 256], bf16,
                            kind="ExternalInput").ap()
    wv_d = nc.dram_tensor("wv", [128, KT * 256], bf16,
                          kind="ExternalInput").ap()
    wp_d = nc.dram_tensor("wp", [128, 2 * C], bf16,
                          kind="ExternalInput").ap()
    bqk_d = nc.dram_tensor("bqk", [512, 1], f32, kind="ExternalInput").ap()
    bv_d = nc.dram_tensor("bv", [128, 4], f32, kind="ExternalInput").ap()
    y_d = nc.dram_tensor("y", [N, C], bf16, kind="ExternalOutput").ap()
    warm_d = nc.dram_tensor("warm", [1, 8], f32, kind="ExternalOutput").ap()

    with tile.TileContext(nc) as tc, ExitStack() as ctx:
        persist = ctx.enter_context(tc.tile_pool(name="persist", bufs=1))
        # PSUM budget (8 banks of 2KB): scores 2x[128,1024]f32 (4 banks) +
        # pv accumulators 2x[128,512] (2) + misc qkv/v/proj chains 2x[128,512]
        # (2). Dedicated pools keep the score rotation independent of the
        # qkv/proj chains so the exp stream starts as soon as k0/q0 land.
        s_pool = ctx.enter_context(
            tc.tile_pool(name="s", bufs=2, space="PSUM"))
        pv_pool = ctx.enter_context(
            tc.tile_pool(name="pv", bufs=2, space="PSUM"))
        mm_pool = ctx.enter_context(
            tc.tile_pool(name="mm", bufs=2, space="PSUM"))
        es_pool = ctx.enter_context(tc.tile_pool(name="es", bufs=6))
        z_pool = ctx.enter_context(tc.tile_pool(name="z", bufs=4))
        y_pool = ctx.enter_context(tc.tile_pool(name="ysb", bufs=6))
        zd_pool = ctx.enter_context(
            tc.tile_pool(name="zd", bufs=4, space="DRAM"))

        xT = persist.tile([128, KT, N], bf16)
        wqkA = persist.tile([128, KT, 256], bf16)
        wqkB = persist.tile([128, KT, 256], bf16)
        wv = persist.tile([128, KT, 256], bf16)
        wp = persist.tile([128, 2, C], bf16)
        bq = persist.tile([128, 4], f32)
        bv = persist.tile([128, 4], f32) if v_bias_nonzero else None
        # q/k activations split into per-(dim-tile, token-chunk) tiles so the
        # scheduler releases attention matmuls as soon as each chunk lands
        qkT = [[persist.tile([128, 512], bf16, name=f"qkT{nt}_{mc}")
                for mc in range(4)] for nt in range(4)]
        v_sb = persist.tile([128, MT, HL * 65], bf16)
        out_sb = persist.tile([128, 2, N], bf16)
        warm_sb = persist.tile([1, 8], f32)
        scratch = persist.tile([128, 128], bf16)
        ones64 = persist.tile([128, 64], f32)

        def dma_sb(q, dst_ap, src, src_w, kt0, nkt, c0=0, c1=None):
            """One DMA from an SBUF-layout dram tensor (per-kt row width
            src_w) covering kt tiles [kt0, kt0+nkt) x src cols [c0:c1)."""
            c1 = src_w if c1 is None else c1
            q.dma_start(
                dst_ap,
                src.rearrange("p (t c) -> p t c", c=src_w)
                [:, kt0:kt0 + nkt, c0:c1])

        # Input DMAs ordered by consumption deadline, spread over the three
        # DMA queues (sync/scalar/gpsimd -- the only ones with queues;
        # scalar's is free until the exp stream starts). Each dma_start
        # costs ~1us setup + transfer at ~100GB/s per queue, serial per
        # queue, so the critical set is few, large, contiguous transfers.
        # Phase A gates the first score matmuls: bq, wqkA (q01|k01), and
        # xT cols 0:512 (~1.5MB; kt0's chunks lead each queue).
        with tc.high_priority():
            # bqk[512,1] -> [128 partitions, 4 tiles]
            nc.sync.dma_start(bq[:],
                              bqk_d.rearrange("(t p) o -> p (t o)", p=128))
            if v_bias_nonzero:
                # bv[128, 4]: col h = bias of head h (d at p%64, doubled
                # across both partition halves)
                nc.scalar.dma_start(bv[:], bv_d[:])
            dma_sb(nc.sync, xT[:, 0:2, 0:512], xTf_d, 512, 0, 2)
            dma_sb(nc.scalar, xT[:, 2:4, 0:512], xTf_d, 512, 2, 2)
            dma_sb(nc.gpsimd, wqkA[:, 0:4, :], wqkA_d, 256, 0, 4)
            dma_sb(nc.sync, xT[:, 4:6, 0:512], xTf_d, 512, 4, 2)
            dma_sb(nc.scalar, xT[:, 6:8, 0:512], xTf_d, 512, 6, 2)
            dma_sb(nc.gpsimd, wqkA[:, 4:8, :], wqkA_d, 256, 4, 4)

            # warm-up exp: pulls the ACT table load off the critical path
            nc.vector.memset(warm_sb[:], 0.0)
            nc.scalar.activation(warm_sb[:], warm_sb[:], Exp)
            nc.sync.dma_start(warm_d[:], warm_sb[:])
            # ones columns of v_aug (per head, col 64 of each 65-col group)
            ones_ap = v_sb[:].rearrange("p mt (h c) -> p (mt h) c", c=65)
            nc.vector.memset(ones_ap[:, :, 64:65], 1.0)
            nc.vector.memset(ones64[:], 1.0)

            nc.vector.memset(scratch[:], 0.0)

        # Phase B: wv (gates the v chains from ~step 0 of round 0) and
        # xT cols 512:1024 (k chunk 1, scores from step 2; v tiles 4-7).
        dma_sb(nc.gpsimd, wv[:, 0:4, :], wv_d, 256, 0, 4)
        dma_sb(nc.sync, xT[:, 0:2, 512:1024], xTr_d, 1536, 0, 2, 0, 512)
        dma_sb(nc.scalar, xT[:, 4:6, 512:1024], xTr_d, 1536, 4, 2, 0, 512)
        dma_sb(nc.gpsimd, wv[:, 4:8, :], wv_d, 256, 4, 4)
        dma_sb(nc.sync, xT[:, 2:4, 512:1024], xTr_d, 1536, 2, 2, 0, 512)
        dma_sb(nc.scalar, xT[:, 6:8, 512:1024], xTr_d, 1536, 6, 2, 0, 512)
        # Phase C: xT cols 1024:1536 (k chunk 2 / v tiles 8-11)
        dma_sb(nc.gpsimd, xT[:, 0:2, 1024:1536], xTr_d, 1536, 0, 2, 512,
               1024)
        dma_sb(nc.sync, xT[:, 2:4, 1024:1536], xTr_d, 1536, 2, 2, 512, 1024)
        dma_sb(nc.scalar, xT[:, 4:6, 1024:1536], xTr_d, 1536, 4, 2, 512,
               1024)
        dma_sb(nc.gpsimd, xT[:, 6:8, 1024:1536], xTr_d, 1536, 6, 2, 512,
               1024)
        # Phase D: xT cols 1536:2048, wqkB (q23/k23, rounds 4-7), wp
        # (projection, from round 4 on)
        dma_sb(nc.sync, xT[:, 0:2, 1536:2048], xTr_d, 1536, 0, 2, 1024, 1536)
        dma_sb(nc.scalar, xT[:, 4:6, 1536:2048], xTr_d, 1536, 4, 2, 1024,
               1536)
        dma_sb(nc.gpsimd, wqkB[:, 0:4, :], wqkB_d, 256, 0, 4)
        dma_sb(nc.sync, xT[:, 2:4, 1536:2048], xTr_d, 1536, 2, 2, 1024, 1536)
        dma_sb(nc.scalar, xT[:, 6:8, 1536:2048], xTr_d, 1536, 6, 2, 1024,
               1536)
        dma_sb(nc.gpsimd, wqkB[:, 4:8, :], wqkB_d, 256, 4, 4)
        dma_sb(nc.gpsimd, wp[:], wp_d, C, 0, 2)

        # wqkA holds [q01|k01], wqkB holds [q23|k23]; nt 0..3 =
        # q01,q23,k01,k23 as before
        W_OFF = {0: (0, 0), 2: (0, 128), 1: (1, 256), 3: (1, 384)}

        def qk_block(nt, mcs=range(4)):
            half, off = W_OFF[nt]
            wt = wqkA if half == 0 else wqkB
            for mc in mcs:
                ps = mm_pool.tile([128, 512], f32, tag="mm",
                                  name=f"qk{nt}_{mc}")
                for kt in range(KT):
                    nc.tensor.matmul(
                        ps[:],
                        lhsT=wt[:, kt, off % 256:off % 256 + 128],
                        rhs=xT[:, kt, mc * 512:(mc + 1) * 512],
                        start=(kt == 0), stop=(kt == KT - 1))
                nc.vector.tensor_scalar(
                    out=qkT[nt][mc][:], in0=ps[:],
                    scalar1=bq[:, off // 128:off // 128 + 1],
                    scalar2=None, op0=add)

        def v_block(mts):
            for mt in mts:
                ps = mm_pool.tile([128, 256], f32, tag="mm", name=f"v{mt}")
                for kt in range(KT):
                    nc.tensor.matmul(
                        ps[:],
                        lhsT=xT[:, kt, mt * 128:(mt + 1) * 128],
                        rhs=wv[:, kt, :],
                        start=(kt == 0), stop=(kt == KT - 1))
                # v_aug per head = [v | ones]: the PV matmul then puts v at
                # psum partitions 0..63 and the denominator Z at partition 64
                dst = v_sb[:, mt, :].rearrange("p (h c) -> p h c", c=65)
                nc.vector.tensor_copy(
                    dst[:, :, 0:64], ps[:].rearrange("p (h c) -> p h c",
                                                     c=64))

        NG = MT

        def s_group(step):
            rnd, jt = step // NG, step % NG
            hp, ic = rnd // 4, rnd % 4
            ss = s_pool.tile([128, 1024], f32, tag="s",
                             name=f"s{hp}_{ic}_{jt}")
            for hh in range(2):
                po = hh * 64
                nc.tensor.matmul(
                    ss[:, hh * 512:(hh + 1) * 512],
                    lhsT=qkT[2 + hp][jt // 4][
                        po:po + 64, (jt % 4) * 128:(jt % 4 + 1) * 128],
                    rhs=qkT[hp][ic][po:po + 64, :],
                    start=True, stop=True)
            return ss

        def pv_normalize(hp, ic, pvs, fast=False):
            # in the tail, head 1's chain (which ends in a cross-partition
            # DMA) is the critical path -- start it first
            for hh in ((1, 0) if fast else (0, 1)):
                # release the pv psum slot quickly with a single copy, then
                # run the whole normalize chain from SBUF off-critical-path.
                # Even head: data at partitions 0:64, Z at 64. Odd head:
                # data at 64:128, Z at 63. The Z broadcast goes through a
                # DRAM round-trip normally; in the tail (fast=True) a K=1
                # fp32 matmul on the then-idle PE does it with ~5us less
                # latency.
                pv = pvs[hh]
                zb = z_pool.tile([64, 512], f32, tag="zb")
                if fast:
                    # tail path: ACT (idle after the last exp) lifts the Z
                    # row to SBUF, a K=1 fp32 matmul broadcasts it, and the
                    # normalize multiply reads the pv psum directly -- no
                    # oa round-trip, ~4us less latency.
                    oa = z_pool.tile([128, 512], f32, tag="oa")
                    nc.scalar.copy(oa[64:65, :], pv[64:65, :])
                    zp = s_pool.tile([128, 1024], f32, tag="s",
                                     name=f"zbc{hp}_{ic}_{hh}")
                    nc.tensor.matmul(zp[0:64, 0:512],
                                     lhsT=ones64[64:65, :],
                                     rhs=oa[64:65, :],
                                     start=True, stop=True)
                    nc.vector.reciprocal_approx_fast(zb[:], zp[0:64, 0:512])
                    src = pv
                else:
                    oa = z_pool.tile([128, 512], f32, tag="oa")
                    nc.vector.tensor_copy(oa[:], pv[:])
                    zd = zd_pool.tile([1, 512], f32, tag="zd")
                    nc.sync.dma_start(zd[:], oa[64:65, :])
                    zbz = z_pool.tile([64, 512], f32, tag="zbz")
                    nc.sync.dma_start(
                        zbz[:], zd[0:1, :].to_broadcast([64, 512]))
                    nc.vector.reciprocal_approx_fast(zb[:], zbz[:])
                    src = oa
                if hh == 0:
                    dst = out_sb[0:64, hp, ic * 512:(ic + 1) * 512]
                else:
                    dst = z_pool.tile([64, 512], bf16, tag="o1")
                nc.vector.tensor_mul(dst, src[0:64, :], zb[:])
                if v_bias_nonzero:
                    h = 2 * hp + hh
                    nc.vector.tensor_scalar(
                        out=dst, in0=dst, scalar1=bv[0:64, h:h + 1],
                        scalar2=None, op0=add)
                if hh == 1:
                    # cross-partition move to out_sb[64:128]; the scalar
                    # DMA queue is idle in the tail
                    q = nc.scalar if fast else nc.sync
                    q.dma_start(
                        out_sb[64:128, hp, ic * 512:(ic + 1) * 512],
                        dst[:])

        out_q = [nc.sync, nc.gpsimd]

        def proj_block(its):
            for it in its:
                for oc in range(2):
                    ps = mm_pool.tile([128, 512], f32, tag="mm",
                                      name=f"y{it}_{oc}")
                    for ct in range(2):
                        nc.tensor.matmul(
                            ps[:],
                            lhsT=out_sb[:, ct, it * 128:(it + 1) * 128],
                            rhs=wp[:, ct, oc * 512:(oc + 1) * 512],
                            start=(ct == 0), stop=(ct == 1))
                    ysb = y_pool.tile([128, 512], bf16, tag="y")
                    nc.vector.tensor_copy(ysb[:], ps[:])
                    out_q[(2 * it + oc) % 2].dma_start(
                        y_d[it * 128:(it + 1) * 128,
                            oc * 512:(oc + 1) * 512],
                        ysb[:])

        def proj_tail():
            # Last i-chunk: its ct=0 half (heads 0,1; ready since round
            # (0,3)) runs during the final normalize latency; ct=1 lands
            # as soon as out_sb ct1 is written. Two waves over 4 psum
            # slots (mm + the now-free pv pool); drains split DVE/ACT;
            # y DMAs on the idle scalar queue plus sync/gpsimd.
            tq = [nc.scalar, nc.sync, nc.gpsimd]
            for wave in ((12, 13), (14, 15)):
                units = [(it, oc) for it in wave for oc in range(2)]
                pss = []
                for k, (it, oc) in enumerate(units):
                    pool, tag = ((mm_pool, "mm") if k % 2 == 0
                                 else (pv_pool, "pv"))
                    ps = pool.tile([128, 512], f32, tag=tag,
                                   name=f"yt{it}_{oc}")
                    nc.tensor.matmul(
                        ps[:], lhsT=out_sb[:, 0, it * 128:(it + 1) * 128],
                        rhs=wp[:, 0, oc * 512:(oc + 1) * 512],
                        start=True, stop=False)
                    pss.append(ps)
                for k, (it, oc) in enumerate(units):
                    ps = pss[k]
                    nc.tensor.matmul(
                        ps[:], lhsT=out_sb[:, 1, it * 128:(it + 1) * 128],
                        rhs=wp[:, 1, oc * 512:(oc + 1) * 512],
                        start=False, stop=True)
                    ysb = y_pool.tile([128, 512], bf16, tag="y")
                    if k % 2 == 1:
                        nc.scalar.copy(ysb[:], ps[:])
                    else:
                        nc.vector.tensor_copy(ysb[:], ps[:])
                    tq[k % 3].dma_start(
                        y_d[it * 128:(it + 1) * 128,
                            oc * 512:(oc + 1) * 512],
                        ysb[:])

        # Critical path to the first exp: k and q of heads 0,1 for tokens
        # 0:512 (j-tiles 0-3, i-chunk 0).
        qk_block(2, [0])           # k chunk 0 for heads 0,1
        qk_block(0, [0])           # q chunk 0 for heads 0,1
        # Everything else is PE filler under the exp stream, ordered by
        # when round 0 needs it: v tiles jt feed PV step jt, k chunk c
        # feeds score steps 4c.., q chunks feed later rounds.
        # deadline order (in exp steps): v[jt] -> step jt, k chunk c ->
        # step 4c-2 (scores run LOOK ahead), q0[ic] -> step 16ic-2,
        # k3/q1 -> rounds 4-7
        with tc.high_priority(offset=-20000):
            v_block(range(0, 2))
            qk_block(2, [1])
            v_block(range(2, 6))
            qk_block(2, [2])
            v_block(range(6, 10))
            qk_block(2, [3])
            v_block(range(10, 13))
            qk_block(0, [1])
            v_block(range(13, 16))
            qk_block(0, [2])
            qk_block(0, [3])
            qk_block(3, [0])
            qk_block(1, [0])
            qk_block(3, [1])
            qk_block(3, [2])
            qk_block(3, [3])
            qk_block(1, [1])
            qk_block(1, [2, 3])

        # One flat software pipeline across all 8 (hp, ic) rounds: scores
        # stay LOOK groups ahead of the exp stream so the in-order PE queue
        # never head-of-line-blocks it.
        NSTEP = 8 * NG
        LOOK = 2
        def pv_step(pvs, hp, jt, es):
            for hh in range(2):
                h = 2 * hp + hh
                nc.tensor.matmul(
                    pvs[hh][0:65, :],
                    lhsT=v_sb[:, jt, h * 65:(h + 1) * 65],
                    rhs=es[:, hh * 512:(hh + 1) * 512],
                    start=(jt == 0), stop=(jt == MT - 1))

        with tc.high_priority():
            ss_q = {i: s_group(i) for i in range(LOOK)}
            pvs = None
            es0 = None
            for st in range(NSTEP):
                rnd, jt = st // NG, st % NG
                hp, ic = rnd // 4, rnd % 4
                if jt == 0:
                    pvs = [pv_pool.tile([128, 512], f32, tag="pv",
                                        name=f"pv{hp}_{ic}_{i}")
                           for i in range(2)]
                es = es_pool.tile([128, 1024], bf16, tag="es")
                nc.scalar.activation(es[:], ss_q[st % LOOK][:], Exp,
                                     scale=SCALE)
                if st + LOOK < NSTEP:
                    ss_q[st % LOOK] = s_group(st + LOOK)
                # jt==0's PV matmuls wait on the pv slot being drained
                # (previous round's oa copy); defer their PE-queue slot by
                # one step so they don't head-of-line-block the next exp's
                # scores at the round boundary
                if jt == 0:
                    es0 = es
                else:
                    if jt == 1:
                        pv_step(pvs, hp, 0, es0)
                    pv_step(pvs, hp, jt, es)
                if jt == NG - 1:
                    pv_normalize(hp, ic, pvs, fast=(st == NSTEP - 1))
                    if hp == 1:
                        # both head-pairs of i-chunk ic done: its output
                        # projection becomes pure filler -- except the last
                        # chunk, which IS the tail critical path
                        if ic == 3:
                            proj_tail()
                        else:
                            with tc.high_priority(offset=-15000):
                                proj_block(range(4 * ic, 4 * ic + 4))

    nc.compile()
    return nc


def _prep_inputs(x, w_qkv, b_qkv, w_proj):
    """Build the 8 per-core input maps (host-side shard + transpose + cast)."""
    w3 = w_qkv.reshape(C, 3, H, D)
    b3 = b_qkv.reshape(3, H, D)
    in_maps = []
    for c in range(N_CORES):
        b, g = divmod(c, 4)
        hs = slice(g * HL, (g + 1) * HL)
        wq = w3[:, 0, hs, :].reshape(C, 256)
        wk = w3[:, 1, hs, :].reshape(C, 256)
        wvl = w3[:, 2, hs, :].reshape(C, 256)
        bqh = b3[0, hs, :].reshape(256)
        bkh = b3[1, hs, :].reshape(256)
        bvh = b3[2, hs, :].reshape(256)
        # q/k transposed layout: head pair (2j, 2j+1) shares an SBUF tile
        # with partition offsets 0/64. All matrices are pre-arranged in
        # SBUF layout [128, kt*cols] so device DMAs are contiguous.
        def sb_layout(m, cols):
            return np.ascontiguousarray(
                m.reshape(KT, 128, cols).transpose(1, 0, 2)
                .reshape(128, KT * cols)).astype(BF)

        xt = x[b].T                       # [C, N]
        in_maps.append({
            "xTf": sb_layout(xt[:, 0:512], 512),
            "xTr": sb_layout(xt[:, 512:], N - 512),
            "wqkA": sb_layout(
                np.concatenate([wq[:, :128], wk[:, :128]], axis=1), 256),
            "wqkB": sb_layout(
                np.concatenate([wq[:, 128:], wk[:, 128:]], axis=1), 256),
            "wv": sb_layout(wvl, 256),
            "wp": np.ascontiguousarray(
                w_proj[g * 256:(g + 1) * 256, :].reshape(2, 128, C)
                .transpose(1, 0, 2).reshape(128, 2 * C)).astype(BF),
            "bqk": np.concatenate(
                [bqh[:128], bkh[:128], bqh[128:], bkh[128:]])
                .reshape(512, 1).astype(np.float32),
            "bv": np.ascontiguousarray(
                      np.tile(bvh.reshape(4, 64).T, (2, 1)))
                    .astype(np.float32),
        })
    return in_maps


def _get_program(v_bias_nonzero: bool):
    key = ("prog", v_bias_nonzero)
    if key not in _cache:
        _install_ntff_hook()
        _cache[key] = _build_program(v_bias_nonzero)
    return _cache[key]


def run(x, w_qkv, b_qkv, w_proj, b_proj, trace=False, trace_kwargs=None):
    from concourse import bass_utils
    bass_utils.upload_artifacts = lambda tmpdir: tmpdir  # no cloud upload

    x = np.asarray(x, dtype=np.float32)
    w_qkv = np.asarray(w_qkv, dtype=np.float32)
    b_qkv = np.asarray(b_qkv, dtype=np.float32)
    w_proj = np.asarray(w_proj, dtype=np.float32)
    b_proj = np.asarray(b_proj, dtype=np.float32)

    v_bias_nonzero = bool(np.any(b_qkv.reshape(3, H, D)[2] != 0.0))
    nc = _get_program(v_bias_nonzero)
    in_maps = _prep_inputs(x, w_qkv, b_qkv, w_proj)
    res = bass_utils.run_bass_kernel_spmd(
        nc, in_maps, list(range(N_CORES)), trace=trace,
        **(trace_kwargs or {}))

    out = np.zeros((B, N, C), dtype=np.float32)
    for b in range(B):
        acc = np.zeros((N, C), dtype=np.float32)
        for g in range(4):
            acc += np.asarray(res.results[b * 4 + g]["y"],
                              dtype=np.float32)
        out[b] = acc + b_proj
    return out, res


def kernel(x, w_qkv, b_qkv, w_proj, b_proj):
    out, _ = run(x, w_qkv, b_qkv, w_proj, b_proj, trace=False)
    return out
